# revision 1
# baseline (speedup 1.0000x reference)
"""GCNEncoder (GCNConv + TransformerEncoderLayer) on 8 Trainium2 NeuronCores.

Sharding: nodes are split 512/core (8 cores). Per core:
  - GCN: dense normalized-adjacency block A^T [4096 src, 512 dst] built on
    device via GPSIMD local_scatter from host-permuted (index-only) edge
    layouts; aggregation is a dense fp16 matmul against the AllGathered
    scaled features.
  - Attention: both heads, q = the core's 512 nodes vs all 4096 keys.
    Scores computed transposed (S^T[k,q]) so softmax denominators come from
    a ones-matmul and PV needs no transposes; softmax skips max-subtraction
    (scores are O(1) for this model family; exp cannot overflow fp32).
  - FFN + both LayerNorms fully local.
Two AllGathers (scaled GCN features, hidden-state transpose) are the only
collectives. All matmul operands fp16, accumulation fp32 in PSUM.
"""

import math

import numpy as np

import concourse.bacc as bacc
import concourse.mybir as mybir
import concourse.tile as tile
from concourse import library_config
from concourse.tile_rust import add_dep_helper

N_CORES = 8
N = 4096
E = 131072
DIN = 512
D = 256
H = 2
DH = 128
DFF = 2048
EPS = 1e-5
P = 128

NPC = N // N_CORES          # nodes per core = 512
MPC = NPC // P              # m-chunks per core = 4
KT = N // P                 # src k-tiles = 32
KPAD = 32                   # max out-edges per (core, src-node)
KBD = 80                    # max in-edges per dst node
NDUP = 256                  # max duplicate-edge occurrences per core
DT16 = mybir.dt.float16
DT32 = mybir.dt.float32
DTI16 = mybir.dt.int16
F = mybir.ActivationFunctionType
A = mybir.AluOpType
INV_SQRT_DH = 1.0 / math.sqrt(DH)


def build_kernel():
    nc = bacc.Bacc("TRN2", target_bir_lowering=False, debug=False,
                   num_devices=N_CORES)

    def din(name, shape, dt=DT32):
        return nc.dram_tensor(name, shape, dt, kind="ExternalInput")

    xT_d = din("xT", [P, MPC * DIN], DT16)
    xTf_d = din("xTf", [P, (DIN // P) * N], DT16)   # full x.T wrapped
    wbdf_d = din("wbdf", [P, (N // P) * KBD], DT16)  # full per-dst weights
    wg_d = din("wg", [P, (DIN // P) * D], DT16)
    warr_d = din("warr", [P, KT * KPAD], DT16)
    idx_d = din("idx", [P, KT * KPAD], DTI16)
    wbd_d = din("wbd", [P, MPC * KBD], DT16)
    dupsr_d = din("dupsr", [P, NDUP // P])
    dupfc_d = din("dupfc", [P, NDUP // P])
    dupw_d = din("dupw", [P, NDUP // P])
    iota1024_d = din("iota1024", [P, KT * KPAD])
    iota128_d = din("iota128", [P, P])
    ident_d = din("ident", [P, P])
    winT_d = din("winT", [P, 2 * 3 * D], DT16)
    ipb_d = din("ipb", [P, 6])
    woT_d = din("woT", [P, 2 * D], DT16)
    w1T_d = din("w1T", [P, 2 * DFF], DT16)
    b1_d = din("b1", [P, DFF // P])
    w2T_d = din("w2T", [P, (DFF // P) * D], DT16)
    bias_d = din("bias", [1, 7 * D])

    out_d = nc.dram_tensor("out", [NPC, D], DT32, kind="ExternalOutput")

    with tile.TileContext(nc) as tc:
        with (
            tc.tile_pool(name="keep", bufs=1) as keep,
            tc.tile_pool(name="dram", bufs=1, space="DRAM") as dram,
        ):
            def load16(dram_t, cols):
                f16 = keep.tile([P, cols], DT16, tag=f"ld_{dram_t.name}",
                                name=f"{dram_t.name}16")
                nc.sync.dma_start(f16[:], dram_t[:])
                return f16

            def bc4(ap_2d):
                """[128, D] bias slice -> broadcast [128, MPC, D]."""
                return ap_2d[:, None, :].to_broadcast([P, MPC, D])

            ones16_col = keep.tile([P, 1], DT16)
            ones16_row = keep.tile([1, P], DT16)
            ones32_row = keep.tile([1, P], DT32)
            nc.vector.memset(ones16_col[:], 1.0)
            nc.vector.memset(ones16_row[:], 1.0)
            nc.vector.memset(ones32_row[:], 1.0)

            lib = nc.gpsimd.load_library(library_config.local_scatter)

            gk = ctx_gcn = tc.tile_pool(name="gcn_keep", bufs=1)
            gk = ctx_gcn.__enter__()

            # ---- A build first: scatters on GpSimd start ASAP ----
            iota1024 = gk.tile([P, KT * KPAD], DT32)
            iota128 = gk.tile([P, P], DT32)
            warr = gk.tile([P, KT * KPAD], DT16)
            idx_t = gk.tile([P, KT * KPAD], DTI16)
            dupsr = gk.tile([P, NDUP // P], DT32)
            dupfc = gk.tile([P, NDUP // P], DT32)
            dupw = gk.tile([P, NDUP // P], DT32)
            nc.sync.dma_start(warr[:], warr_d[:])
            nc.sync.dma_start(idx_t[:], idx_d[:])
            nc.sync.dma_start(iota1024[:], iota1024_d[:])
            nc.sync.dma_start(iota128[:], iota128_d[:])
            nc.sync.dma_start(dupsr[:], dupsr_d[:])
            nc.sync.dma_start(dupfc[:], dupfc_d[:])
            nc.sync.dma_start(dupw[:], dupw_d[:])

            warr16 = gk.tile([P, KT * KPAD], DT16)
            a_tiles = [gk.tile([P, NPC], DT16, tag=f"A{kt}", name=f"A{kt}")
                       for kt in range(KT)]

            with tc.tile_pool(name="gcn_sb", bufs=2) as gsb, \
                 tc.tile_pool(name="gcn_ps", bufs=2, space="PSUM") as gps:
                mrg_ps = [gps.tile([P, 512], DT32, space="PSUM",
                                   tag=f"mrg{h}", name=f"mrg{h}")
                          for h in range(2)]
                for b in range(NDUP // P):
                    sd = gsb.tile([P, P], DT16, tag="sd")
                    vd = gsb.tile([P, KT * KPAD], DT16, tag="vd")
                    nc.vector.tensor_scalar(sd[:], iota128[:],
                                            dupsr[:, b:b + 1], None,
                                            op0=A.is_equal)
                    nc.vector.tensor_scalar(vd[:], iota1024[:],
                                            dupfc[:, b:b + 1],
                                            dupw[:, b:b + 1],
                                            op0=A.is_equal, op1=A.mult)
                    for h in range(2):
                        nc.tensor.matmul(mrg_ps[h][:], lhsT=sd[:],
                                         rhs=vd[:, 512 * h:512 * h + 512],
                                         start=(b == 0),
                                         stop=(b == NDUP // P - 1))
                for h in range(2):
                    nc.vector.tensor_tensor(warr16[:, 512 * h:512 * h + 512],
                                            warr[:, 512 * h:512 * h + 512],
                                            mrg_ps[h][:], op=A.add)
                last_scatter = None
                for kt in range(KT):
                    ls = nc.gpsimd.local_scatter(
                        a_tiles[kt][:],
                        warr16[:, KPAD * kt:KPAD * (kt + 1)],
                        idx_t[:, KPAD * kt:KPAD * (kt + 1)],
                        channels=P, num_elems=NPC, num_idxs=KPAD,
                    )
                    add_dep_helper(ls.ins, lib.ins, reason="scatter after lib")
                    last_scatter = ls

            # ---- degrees -> dinv (local + full) ----
            wbd = gk.tile([P, MPC * KBD], DT16)
            nc.sync.dma_start(wbd[:], wbd_d[:])
            dinv = gk.tile([P, MPC], DT32)
            dinv2 = gk.tile([P, MPC], DT32)
            deg = gk.tile([P, MPC], DT32)
            nc.vector.tensor_reduce(
                deg[:], wbd[:].rearrange("p (m k) -> p m k", k=KBD),
                axis=mybir.AxisListType.X, op=A.add)
            sqd = gk.tile([P, MPC], DT32)
            nc.scalar.activation(sqd[:], deg[:], F.Sqrt, bias=1.0, scale=1.0)
            nc.vector.reciprocal(dinv[:], sqd[:])
            nc.vector.tensor_mul(dinv2[:], dinv[:], dinv[:])

            wbdf = gk.tile([P, (N // P) * KBD], DT16)
            nc.sync.dma_start(wbdf[:], wbdf_d[:])
            dinvf = gk.tile([P, N // P], DT32)
            degf = gk.tile([P, N // P], DT32)
            nc.vector.tensor_reduce(
                degf[:], wbdf[:].rearrange("p (j k) -> p j k", k=KBD),
                axis=mybir.AxisListType.X, op=A.add)
            sqdf = gk.tile([P, N // P], DT32)
            nc.scalar.activation(sqdf[:], degf[:], F.Sqrt, bias=1.0, scale=1.0)
            nc.vector.reciprocal(dinvf[:], sqdf[:])

            # ---- xw = x @ W_gcn: full (replicated) + local self-term ----
            xT16 = load16(xT_d, MPC * DIN)
            wg16 = load16(wg_d, (DIN // P) * D)
            xTf16 = gk.tile([P, (DIN // P) * N], DT16)
            nc.sync.dma_start(xTf16[:], xTf_d[:])
            xws16f = gk.tile([P, (N // P) * D], DT16)
            self32 = gk.tile([P, MPC * D], DT32)
            with tc.tile_pool(name="xw_ps", bufs=4, space="PSUM") as xps:
                for m in range(MPC):
                    pxw = xps.tile([P, D], DT32, space="PSUM", tag="xw")
                    for k in range(DIN // P):
                        nc.tensor.matmul(
                            pxw[:],
                            lhsT=xT16[:, DIN * k + P * m:DIN * k + P * m + P],
                            rhs=wg16[:, D * k:D * (k + 1)],
                            start=(k == 0), stop=(k == DIN // P - 1))
                    nc.vector.tensor_scalar(self32[:, D * m:D * (m + 1)], pxw[:],
                                            dinv2[:, m:m + 1], None, op0=A.mult)
                for j in range(N // P):
                    pxw = xps.tile([P, D], DT32, space="PSUM", tag="xw")
                    for k in range(DIN // P):
                        nc.tensor.matmul(
                            pxw[:],
                            lhsT=xTf16[:, N * k + P * j:N * k + P * (j + 1)],
                            rhs=wg16[:, D * k:D * (k + 1)],
                            start=(k == 0), stop=(k == DIN // P - 1))
                    nc.vector.tensor_scalar(xws16f[:, D * j:D * (j + 1)],
                                            pxw[:], dinvf[:, j:j + 1], None,
                                            op0=A.mult)

            # constants for later phases (DMA after critical ones)
            ident = keep.tile([P, P], DT32)
            ipb = keep.tile([P, 6], DT32)
            b1t = keep.tile([P, DFF // P], DT32)
            nc.sync.dma_start(ident[:], ident_d[:])
            nc.sync.dma_start(ipb[:], ipb_d[:])
            nc.sync.dma_start(b1t[:], b1_d[:])
            winT16 = load16(winT_d, 2 * 3 * D)
            woT16 = load16(woT_d, 2 * D)

            bias_row = keep.tile([1, 7 * D], DT32)
            nc.sync.dma_start(bias_row[:], bias_d[:])
            ipb16 = keep.tile([P, 6], DT16)
            nc.vector.tensor_copy(ipb16[:], ipb[:])
            bias_bc = keep.tile([P, 7 * D], DT32)
            with tc.tile_pool(name="ps_b", bufs=2, space="PSUM") as psb:
                for j in range(4):
                    w = 448 if j < 3 else 7 * D - 3 * 448
                    pb = psb.tile([P, 448], DT32, space="PSUM", tag="bb")
                    nc.tensor.matmul(pb[:, :w], lhsT=ones32_row[:],
                                     rhs=bias_row[:, j * 448:j * 448 + w],
                                     start=True, stop=(j != 3))
                    if j == 3:
                        # softmax rows sum to 1, so the V bias contributes the
                        # constant (concat_h bv_h) @ W_o^T — accumulate it
                        # onto out_proj_b in the broadcast tile.
                        for h in range(H):
                            nc.tensor.matmul(
                                pb[:, 192:448],
                                lhsT=ipb16[:, 4 + h:5 + h].to_broadcast([P, P]),
                                rhs=woT16[:, D * h:D * (h + 1)],
                                start=False, stop=(h == H - 1))
                    nc.vector.tensor_copy(bias_bc[:, j * 448:j * 448 + w],
                                          pb[:, :w])
            bgcn_bc = bias_bc[:, 0:D]
            b2_bc = bias_bc[:, D:2 * D]
            ln1g_bc = bias_bc[:, 2 * D:3 * D]
            ln1b_bc = bias_bc[:, 3 * D:4 * D]
            ln2g_bc = bias_bc[:, 4 * D:5 * D]
            ln2b_bc = bias_bc[:, 5 * D:6 * D]
            bo_bc = bias_bc[:, 6 * D:7 * D]

            # ---- aggregation ----
            h_t = keep.tile([P, MPC * D], DT32)
            hT16 = keep.tile([P, 2 * NPC], DT16)
            with tc.tile_pool(name="agg_sb", bufs=1) as asb, \
                 tc.tile_pool(name="agg_ps", bufs=1, space="PSUM") as aps:
                agg_ps = [aps.tile([P, D], DT32, space="PSUM",
                                   tag=f"agg{m}", name=f"agg{m}")
                          for m in range(MPC)]
                for kt in range(KT):
                    for m in range(MPC):
                        agg_mm = nc.tensor.matmul(
                            agg_ps[m][:],
                            lhsT=a_tiles[kt][:, P * m:P * (m + 1)],
                            rhs=xws16f[:, D * kt:D * (kt + 1)],
                            start=(kt == 0), stop=(kt == KT - 1))
                        if kt == 0:
                            # single barrier: stream all 128 agg matmuls after
                            # the last scatter instead of trickling per-tile
                            add_dep_helper(agg_mm.ins, last_scatter.ins,
                                           reason="agg after all scatters")

                # h = relu(dinv*agg + self + b_gcn)   (batched epilogue)
                x_all = asb.tile([P, MPC * D], DT32, tag="xall")
                for m in range(MPC):
                    nc.vector.scalar_tensor_tensor(
                        x_all[:, D * m:D * (m + 1)], agg_ps[m][:],
                        dinv[:, m:m + 1], self32[:, D * m:D * (m + 1)],
                        op0=A.mult, op1=A.add)
                nc.vector.tensor_tensor(
                    x_all[:].rearrange("p (m d) -> p m d", m=MPC),
                    x_all[:].rearrange("p (m d) -> p m d", m=MPC),
                    bc4(bgcn_bc), op=A.add)
                nc.scalar.activation(h_t[:], x_all[:], F.Relu)

            # transpose h -> hT16 (local feature-major)
            with tc.tile_pool(name="tr_ps", bufs=2, space="PSUM") as tps:
                for m in range(MPC):
                    for f in range(2):
                        ptr = tps.tile([P, P], DT32, space="PSUM", tag="tr")
                        nc.tensor.transpose(
                            ptr[:], h_t[:, D * m + P * f:D * m + P * (f + 1)],
                            ident[:])
                        nc.vector.tensor_copy(
                            hT16[:, NPC * f + P * m:NPC * f + P * (m + 1)],
                            ptr[:])

            ctx_gcn.__exit__(None, None, None)
            ak = ctx_attn = tc.tile_pool(name="attn_keep", bufs=1)
            ak = ctx_attn.__enter__()

            # ---- local K^T / V / Q^T, then ONE packed KV AllGather ----
            # kv rows: 0:128 K^T h0 | 128:256 K^T h1 | 256:384 V h0 | 384:512 V h1
            # (V packed as [128, m*128+d] = natural [512, 128] per head)
            qT16 = ak.tile([P, H * NPC], DT16)
            kv_sb = ak.tile([P, 4 * NPC], DT16)
            with tc.tile_pool(name="kv_ps", bufs=3, space="PSUM") as kvps:
                for h in range(H):
                    pq = kvps.tile([P, NPC], DT32, space="PSUM", tag="kv")
                    for k in range(2):
                        nc.tensor.matmul(
                            pq[:],
                            lhsT=winT16[:, 768 * k + P * h:768 * k + P * (h + 1)],
                            rhs=hT16[:, NPC * k:NPC * (k + 1)],
                            start=(k == 0), stop=(k == 1))
                    nc.vector.tensor_scalar(
                        qT16[:, NPC * h:NPC * (h + 1)], pq[:],
                        ipb[:, h:h + 1], None, op0=A.add)
                    pk = kvps.tile([P, NPC], DT32, space="PSUM", tag="kv")
                    for k in range(2):
                        nc.tensor.matmul(
                            pk[:],
                            lhsT=winT16[:, 768 * k + D + P * h:
                                        768 * k + D + P * (h + 1)],
                            rhs=hT16[:, NPC * k:NPC * (k + 1)],
                            start=(k == 0), stop=(k == 1))
                    nc.vector.tensor_scalar(
                        kv_sb[:, NPC * h:NPC * (h + 1)], pk[:],
                        ipb[:, 2 + h:3 + h], None, op0=A.add)
                    for m in range(MPC):
                        pv = kvps.tile([P, P], DT32, space="PSUM", tag="kvv")
                        for k in range(2):
                            nc.tensor.matmul(
                                pv[:],
                                lhsT=hT16[:, NPC * k + P * m:NPC * k + P * (m + 1)],
                                rhs=winT16[:, 768 * k + 2 * D + P * h:
                                            768 * k + 2 * D + P * (h + 1)],
                                start=(k == 0), stop=(k == 1))
                        nc.vector.tensor_copy(
                            kv_sb[:, NPC * (2 + h) + P * m:
                                  NPC * (2 + h) + P * (m + 1)], pv[:])

            # FFN weights stream before/while the AllGather runs
            w1T16 = ak.tile([P, 2 * DFF], DT16)
            nc.sync.dma_start(w1T16[:], w1T_d[:])
            w2T16 = ak.tile([P, (DFF // P) * D], DT16)
            nc.sync.dma_start(w2T16[:], w2T_d[:])

            kv_bounce = dram.tile([4 * P, NPC], DT16)
            kv_gath = dram.tile([N_CORES * 4 * P, NPC], DT16,
                                addr_space="Shared")
            nc.scalar.dma_start(
                kv_bounce[:].rearrange("(x p) n -> p x n", p=P),
                kv_sb[:].rearrange("p (x n) -> p x n", x=4))
            nc.gpsimd.collective_compute(
                "AllGather", A.bypass,
                replica_groups=[list(range(N_CORES))],
                ins=[kv_bounce.opt()], outs=[kv_gath.opt()])

            # ---- load gathered K^T / V ----
            kT16 = ak.tile([P, H * N], DT16)
            v16 = ak.tile([P, H * N], DT16)
            gv = kv_gath[:].rearrange("(g x p) n -> x p g n",
                                      g=N_CORES, x=4, p=P)
            for h in range(H):
                nc.scalar.dma_start(
                    kT16[:, N * h:N * (h + 1)].rearrange(
                        "p (g n) -> p g n", g=N_CORES), gv[h])
                nc.scalar.dma_start(
                    v16[:, N * h:N * (h + 1)].rearrange(
                        "p (g n) -> p g n", g=N_CORES), gv[2 + h])

            # ---- S^T -> exp -> PV + sums ----
            oT16 = ak.tile([P, H * NPC], DT16)
            with tc.tile_pool(name="att_sb", bufs=3) as atsb, \
                 tc.tile_pool(name="att_ps", bufs=1, space="PSUM") as atps, \
                 tc.tile_pool(name="s_ps", bufs=2, space="PSUM") as sps:
                o_ps = [atps.tile([P, NPC], DT32, space="PSUM",
                                  tag=f"o{h}", name=f"o{h}")
                        for h in range(H)]
                sum_ps = [atps.tile([1, NPC], DT32, space="PSUM",
                                    tag=f"sm{h}", name=f"sm{h}")
                          for h in range(H)]
                esum = [None, None]
                for kt2 in range(KT // 2):
                    for h in range(H):
                        # two k-tiles of scores into one 2-bank psum; one exp
                        ps_s = sps.tile([P, 2 * NPC], DT32, space="PSUM",
                                        tag="S")
                        for u in range(2):
                            kt = 2 * kt2 + u
                            nc.tensor.matmul(
                                ps_s[:, NPC * u:NPC * (u + 1)],
                                lhsT=kT16[:, N * h + P * kt:N * h + P * (kt + 1)],
                                rhs=qT16[:, NPC * h:NPC * (h + 1)],
                                start=True, stop=True)
                        es = atsb.tile([P, 2 * NPC], DT16, tag="es")
                        nc.scalar.activation(es[:], ps_s[:], F.Exp,
                                             scale=INV_SQRT_DH)
                        for u in range(2):
                            kt = 2 * kt2 + u
                            nc.tensor.matmul(
                                o_ps[h][:],
                                lhsT=v16[:, N * h + P * kt:N * h + P * (kt + 1)],
                                rhs=es[:, NPC * u:NPC * (u + 1)],
                                start=(kt == 0), stop=(kt == KT - 1))
                        if kt2 % 2 == 0:
                            eacc = atsb.tile([P, 2 * NPC], DT16, tag=f"eac{h}",
                                             name=f"eacc{h}")
                            nc.vector.tensor_copy(eacc[:], es[:])
                            esum[h] = eacc
                        else:
                            nc.vector.tensor_add(esum[h][:], esum[h][:], es[:])
                            for u in range(2):
                                nc.tensor.matmul(
                                    sum_ps[h][:], lhsT=ones16_col[:],
                                    rhs=esum[h][:, NPC * u:NPC * (u + 1)],
                                    start=(kt2 == 1 and u == 0),
                                    stop=(kt2 == KT // 2 - 1 and u == 1))

                # copy unnormalized o to sbuf; transpose sums to
                # per-partition [128, MPC] reciprocals
                recT = atsb.tile([P, H * MPC], DT32, tag="recT", name="recT")
                for h in range(H):
                    nc.vector.tensor_copy(oT16[:, NPC * h:NPC * (h + 1)],
                                          o_ps[h][:])
                    srow = atsb.tile([1, NPC], DT32, tag="srow")
                    nc.vector.tensor_copy(srow[:], sum_ps[h][:])
                    sT_ps = sps.tile([P, MPC], DT32, space="PSUM", tag="S",
                                     name="sTps")
                    for m in range(MPC):
                        nc.tensor.transpose(
                            sT_ps[:, m:m + 1], srow[:, P * m:P * (m + 1)],
                            ident[0:1, 0:1])
                    nc.vector.reciprocal(recT[:, MPC * h:MPC * (h + 1)],
                                         sT_ps[:])

            # ---- o_proj + residual + LN1 (batched) ----
            h1_t = ak.tile([P, MPC * D], DT32)
            h1T16 = ak.tile([P, 2 * NPC], DT16)
            with tc.tile_pool(name="ln_sb", bufs=2) as lsb, \
                 tc.tile_pool(name="op_ps", bufs=2, space="PSUM") as ops:

                def layernorm_all(dst, x_all, g_sl, b_sl, tag):
                    """LN over feature dim for all MPC chunks at once.
                    x_all/dst: [128, MPC*D] fp32 tiles."""
                    mu4 = lsb.tile([P, MPC], DT32, tag=f"{tag}mu")
                    nc.vector.tensor_reduce(
                        mu4[:], x_all[:].rearrange("p (m d) -> p m d", m=MPC),
                        axis=mybir.AxisListType.X, op=A.add)
                    negmu4 = lsb.tile([P, MPC], DT32, tag=f"{tag}nm")
                    nc.vector.tensor_scalar(negmu4[:], mu4[:], -1.0 / D, None,
                                            op0=A.mult)
                    sq4 = lsb.tile([P, D], DT32, tag=f"{tag}sq")
                    ssq4 = lsb.tile([P, MPC], DT32, tag=f"{tag}ss")
                    for m in range(MPC):
                        nc.scalar.activation(sq4[:], x_all[:, D * m:D * (m + 1)],
                                             F.Square, bias=negmu4[:, m:m + 1],
                                             accum_out=ssq4[:, m:m + 1])
                    var4 = lsb.tile([P, MPC], DT32, tag=f"{tag}vr")
                    nc.vector.tensor_scalar(var4[:], ssq4[:], 1.0 / D, EPS,
                                            op0=A.mult, op1=A.add)
                    sd4 = lsb.tile([P, MPC], DT32, tag=f"{tag}sd")
                    nc.scalar.activation(sd4[:], var4[:], F.Sqrt)
                    rstd4 = lsb.tile([P, MPC], DT32, tag=f"{tag}rs")
                    nc.vector.reciprocal(rstd4[:], sd4[:])
                    xc = lsb.tile([P, MPC * D], DT32, tag=f"{tag}xc")
                    for m in range(MPC):
                        nc.vector.tensor_scalar(
                            xc[:, D * m:D * (m + 1)], x_all[:, D * m:D * (m + 1)],
                            negmu4[:, m:m + 1], rstd4[:, m:m + 1],
                            op0=A.add, op1=A.mult)
                    nc.vector.tensor_tensor(
                        xc[:].rearrange("p (m d) -> p m d", m=MPC),
                        xc[:].rearrange("p (m d) -> p m d", m=MPC),
                        bc4(g_sl), op=A.mult)
                    nc.vector.tensor_tensor(
                        dst[:].rearrange("p (m d) -> p m d", m=MPC),
                        xc[:].rearrange("p (m d) -> p m d", m=MPC),
                        bc4(b_sl), op=A.add)

                x1_all = lsb.tile([P, MPC * D], DT32, tag="x1all")
                for m in range(MPC):
                    pa = [None, None]
                    for h in range(H):
                        pa[h] = ops.tile([P, D], DT32, space="PSUM", tag="op", name=f"pa{h}")
                        nc.tensor.matmul(
                            pa[h][:],
                            lhsT=oT16[:, NPC * h + P * m:NPC * h + P * (m + 1)],
                            rhs=woT16[:, D * h:D * (h + 1)],
                            start=True, stop=True)
                    t0m = lsb.tile([P, D], DT32, tag="t0m")
                    nc.vector.tensor_scalar(t0m[:], pa[0][:],
                                            recT[:, m:m + 1], None,
                                            op0=A.mult)
                    nc.vector.scalar_tensor_tensor(
                        t0m[:], pa[1][:], recT[:, MPC + m:MPC + m + 1],
                        t0m[:], op0=A.mult, op1=A.add)
                    nc.vector.tensor_add(x1_all[:, D * m:D * (m + 1)], t0m[:],
                                         h_t[:, D * m:D * (m + 1)])
                nc.vector.tensor_tensor(
                    x1_all[:].rearrange("p (m d) -> p m d", m=MPC),
                    x1_all[:].rearrange("p (m d) -> p m d", m=MPC),
                    bc4(bo_bc), op=A.add)
                layernorm_all(h1_t, x1_all, ln1g_bc, ln1b_bc, "a")

                with tc.tile_pool(name="tr2_ps", bufs=2, space="PSUM") as tps2:
                    for m in range(MPC):
                        for f in range(2):
                            ptr = tps2.tile([P, P], DT32, space="PSUM",
                                            tag="tr2")
                            nc.tensor.transpose(
                                ptr[:],
                                h1_t[:, D * m + P * f:D * m + P * (f + 1)],
                                ident[:])
                            nc.vector.tensor_copy(
                                h1T16[:, NPC * f + P * m:NPC * f + P * (m + 1)],
                                ptr[:])

                # ---- FFN ----
                out_sb = ak.tile([P, MPC * D], DT32)
                ff1T = ak.tile([P, (DFF // P) * NPC], DT16)
                with tc.tile_pool(name="f1_ps", bufs=3, space="PSUM") as fps:
                    for dc in range(DFF // P):
                        pf = fps.tile([P, NPC], DT32, space="PSUM", tag="f1")
                        for k in range(2):
                            nc.tensor.matmul(
                                pf[:],
                                lhsT=w1T16[:, DFF * k + P * dc:
                                           DFF * k + P * (dc + 1)],
                                rhs=h1T16[:, NPC * k:NPC * (k + 1)],
                                start=(k == 0), stop=(k == 1))
                        nc.scalar.activation(
                            ff1T[:, NPC * dc:NPC * (dc + 1)], pf[:], F.Relu,
                            bias=b1t[:, dc:dc + 1])

                x2_all = lsb.tile([P, MPC * D], DT32, tag="x2all")
                with tc.tile_pool(name="f2_ps", bufs=2, space="PSUM") as fps2:
                    for m in range(MPC):
                        pf2 = fps2.tile([P, D], DT32, space="PSUM", tag="f2")
                        for kt2 in range(DFF // P):
                            nc.tensor.matmul(
                                pf2[:],
                                lhsT=ff1T[:, NPC * kt2 + P * m:
                                          NPC * kt2 + P * (m + 1)],
                                rhs=w2T16[:, D * kt2:D * (kt2 + 1)],
                                start=(kt2 == 0), stop=(kt2 == DFF // P - 1))
                        nc.vector.scalar_tensor_tensor(
                            x2_all[:, D * m:D * (m + 1)], pf2[:], 1.0,
                            h1_t[:, D * m:D * (m + 1)], op0=A.mult, op1=A.add)
                nc.vector.tensor_tensor(
                    x2_all[:].rearrange("p (m d) -> p m d", m=MPC),
                    x2_all[:].rearrange("p (m d) -> p m d", m=MPC),
                    bc4(b2_bc), op=A.add)
                layernorm_all(out_sb, x2_all, ln2g_bc, ln2b_bc, "b")
                nc.scalar.dma_start(
                    out_d[:].rearrange("(m p) d -> p m d", p=P),
                    out_sb[:].rearrange("p (m d) -> p m d", m=MPC))
            ctx_attn.__exit__(None, None, None)

    nc.compile()
    return nc


# ======================= host-side prep =======================

def _prep_inputs(x, edge_index, edge_weight, W_gcn, b_gcn, in_proj_w,
                 in_proj_b, out_proj_w, out_proj_b, lin1_w, lin1_b, lin2_w,
                 lin2_b, ln1_g, ln1_b, ln2_g, ln2_b):
    """Pure index-permutation / layout prep. Returns per-core input maps."""
    x = np.asarray(x, np.float32)
    src = np.asarray(edge_index[0], np.int64)
    dst = np.asarray(edge_index[1], np.int64)
    w = np.asarray(edge_weight, np.float32)

    def wrap128(a):
        # [n*128, m] -> [128, n*m] with col block t <- rows [128t, 128t+128)
        n = a.shape[0] // P
        return np.ascontiguousarray(
            a.reshape(n, P, a.shape[1]).transpose(1, 0, 2).reshape(P, -1))

    iota1024 = np.tile(np.arange(KT * KPAD, dtype=np.float32), (P, 1))
    iota128 = np.tile(np.arange(P, dtype=np.float32), (P, 1))
    ident = np.eye(P, dtype=np.float32)
    bias_stack = np.concatenate([
        np.asarray(v, np.float32).reshape(-1) for v in
        (b_gcn, lin2_b, ln1_g, ln1_b, ln2_g, ln2_b, out_proj_b)
    ]).reshape(1, -1)

    f16 = np.float16
    shared = {
        "wg": wrap128(np.asarray(W_gcn, np.float32)).astype(f16),
        "iota1024": iota1024, "iota128": iota128,
        "ident": ident,
        "winT": wrap128(np.ascontiguousarray(
            np.asarray(in_proj_w, np.float32).T)).astype(f16),
        "ipb": np.ascontiguousarray(
            np.asarray(in_proj_b, np.float32).reshape(6, P).T),
        "woT": wrap128(np.ascontiguousarray(
            np.asarray(out_proj_w, np.float32).T)).astype(f16),
        "w1T": wrap128(np.ascontiguousarray(
            np.asarray(lin1_w, np.float32).T)).astype(f16),
        "b1": np.ascontiguousarray(
            np.asarray(lin1_b, np.float32).reshape(DFF // P, P).T),
        "w2T": wrap128(np.ascontiguousarray(
            np.asarray(lin2_w, np.float32).T)).astype(f16),
        "bias": bias_stack,
    }

    shared_xTf = wrap128(np.ascontiguousarray(x.T)).astype(f16)
    # full per-dst weight table for replicated degree computation
    wbdf = np.zeros((N, KBD), np.float32)
    cntf = np.zeros(N, np.int32)
    for di, wi in zip(dst.tolist(), w.tolist()):
        j = int(cntf[di])
        assert j < KBD
        wbdf[di, j] = wi
        cntf[di] = j + 1
    wbdf_full_w = wrap128(wbdf).astype(f16)

    core_of = dst // NPC
    in_maps = []
    for c in range(N_CORES):
        sel = np.nonzero(core_of == c)[0]
        s_c = src[sel]
        d_c = (dst[sel] - NPC * c).astype(np.int64)
        w_c = w[sel]

        w_arr = np.zeros((N, KPAD), np.float32)
        idx_arr = np.full((N, KPAD), -1, np.int16)
        counts = np.zeros(N, np.int32)
        first_slot = {}
        dup_sr, dup_fc, dup_w = [], [], []
        for si, di, wi in zip(s_c.tolist(), d_c.tolist(), w_c.tolist()):
            key = si * NPC + di
            slot = first_slot.get(key)
            if slot is None:
                j = int(counts[si])
                assert j < KPAD, f"KPAD overflow at src {si}"
                counts[si] = j + 1
                w_arr[si, j] = wi
                idx_arr[si, j] = di
                first_slot[key] = j
            else:
                dup_sr.append(si % P)
                dup_fc.append(KPAD * (si // P) + slot)
                dup_w.append(wi)
        assert len(dup_sr) <= NDUP, f"NDUP overflow: {len(dup_sr)}"

        def pad_dup(vals, dtype=np.float32):
            a = np.zeros(NDUP, dtype)
            a[:len(vals)] = vals
            return np.ascontiguousarray(a.reshape(NDUP // P, P).T)

        wbd = np.zeros((NPC, KBD), np.float32)
        cnt2 = np.zeros(NPC, np.int32)
        for di, wi in zip(d_c.tolist(), w_c.tolist()):
            j = int(cnt2[di])
            assert j < KBD, f"KBD overflow at dst {di}"
            wbd[di, j] = wi
            cnt2[di] = j + 1

        in_maps.append({
            **shared,
            "xT": wrap128(np.ascontiguousarray(
                x[NPC * c:NPC * (c + 1)].T)).astype(f16),
            "xTf": shared_xTf,
            "wbdf": wbdf_full_w,
            "warr": wrap128(w_arr).astype(f16),
            "idx": wrap128(idx_arr),
            "wbd": wrap128(wbd).astype(f16),
            "dupsr": pad_dup(dup_sr),
            "dupfc": pad_dup(dup_fc),
            "dupw": pad_dup(dup_w),
        })
    return in_maps


# ======================= runner =======================

class _Runner:
    """Persistent-jit SPMD executor (mirrors bass2jax.run_bass_via_pjrt)."""

    def __init__(self, nc):
        import jax
        from jax.sharding import Mesh, PartitionSpec
        from jax.experimental.shard_map import shard_map
        from concourse.bass2jax import (_bass_exec_p, install_neuronx_cc_hook,
                                        partition_id_tensor)
        install_neuronx_cc_hook()
        self.jax = jax
        partition_name = (nc.partition_id_tensor.name
                          if nc.partition_id_tensor else None)
        in_names, out_names, out_avals, zero_outs = [], [], [], []
        for alloc in nc.m.functions[0].allocations:
            if not isinstance(alloc, mybir.MemoryLocationSet):
                continue
            name = alloc.memorylocations[0].name
            if alloc.kind == "ExternalInput":
                if name != partition_name:
                    in_names.append(name)
            elif alloc.kind == "ExternalOutput":
                out_names.append(name)
                shape = tuple(alloc.tensor_shape)
                dtype = mybir.dt.np(alloc.dtype)
                out_avals.append(jax.core.ShapedArray(shape, dtype))
                zero_outs.append(np.zeros(shape, dtype))
        self.in_names, self.out_names = in_names, out_names
        self.out_shapes = [tuple(a.shape) for a in out_avals]
        self.n_params = len(in_names)
        self.zero_outs = zero_outs
        all_in = in_names + out_names
        if partition_name is not None:
            all_in.append(partition_name)

        def _body(*args):
            operands = list(args)
            if partition_name is not None:
                operands.append(partition_id_tensor())
            return tuple(_bass_exec_p.bind(
                *operands, out_avals=tuple(out_avals), in_names=tuple(all_in),
                out_names=tuple(out_names), lowering_input_output_aliases=(),
                sim_require_finite=True, sim_require_nnan=True, nc=nc))

        devices = jax.devices()[:N_CORES]
        self.mesh = Mesh(np.asarray(devices), ("core",))
        nin = self.n_params + len(out_names)
        self.fn = jax.jit(
            shard_map(_body, mesh=self.mesh,
                      in_specs=(PartitionSpec("core"),) * nin,
                      out_specs=(PartitionSpec("core"),) * len(out_names),
                      check_rep=False),
            keep_unused=True)

    def place(self, in_maps):
        import jax
        from jax.sharding import PartitionSpec
        per_core = [[np.asarray(m[n]) for n in self.in_names] for m in in_maps]
        concat = [np.concatenate([per_core[c][i] for c in range(N_CORES)], axis=0)
                  for i in range(self.n_params)]
        zeros = [np.zeros((N_CORES * z.shape[0], *z.shape[1:]), z.dtype)
                 for z in self.zero_outs]
        sh = jax.sharding.NamedSharding(self.mesh, PartitionSpec("core"))
        return [jax.device_put(a, sh) for a in (*concat, *zeros)]

    def run(self, args):
        outs = self.fn(*args)
        self.jax.block_until_ready(outs)
        return outs

    def results(self, outs):
        res = []
        for c in range(N_CORES):
            d = {}
            for i, name in enumerate(self.out_names):
                full = np.asarray(outs[i])
                ps = self.out_shapes[i]
                d[name] = full.reshape((N_CORES,) + ps)[c]
            res.append(d)
        return res


_CACHE = {}


def _get_runner():
    if "runner" not in _CACHE:
        nc = build_kernel()
        _CACHE["nc"] = nc
        _CACHE["runner"] = _Runner(nc)
    return _CACHE["runner"]


def kernel(**inputs) -> np.ndarray:
    runner = _get_runner()
    in_maps = _prep_inputs(**inputs)
    args = runner.place(in_maps)
    outs = runner.run(args)
    res = runner.results(outs)
    return np.concatenate([res[c]["out"] for c in range(N_CORES)], axis=0)



# revision 5
# speedup vs baseline: 1.0336x; 1.0336x over previous
"""GCNEncoder (GCNConv + TransformerEncoderLayer) on 8 Trainium2 NeuronCores.

Sharding: nodes are split 512/core (8 cores). Per core:
  - GCN: dense normalized-adjacency block A^T [4096 src, 512 dst] built on
    device via GPSIMD local_scatter from host-permuted (index-only) edge
    layouts; aggregation is a dense fp16 matmul against replicated scaled
    features, pipelined per src k-tile against scatter completion.
  - Attention: both heads, q = the core's 512 nodes vs all 4096 keys.
    K^T is AllGathered first and S/exp stream against it while the V
    AllGather is still in flight; PV matmuls join the stream once V lands
    (software-pipelined schedule, in-order PE friendly). Softmax skips
    max-subtraction; denominators via one ones-matmul per head over a
    Vector-accumulated exp-sum.
  - FFN + both LayerNorms local, LN pipelined per 128-node chunk.
All rsqrt computed as exp(-0.5*ln(x)) so one activation table serves the
whole kernel. All matmul operands fp16, accumulation fp32 in PSUM.
"""

import math

import numpy as np

import concourse.bacc as bacc
import concourse.mybir as mybir
import concourse.tile as tile
from concourse import library_config
from concourse.tile_rust import add_dep_helper

N_CORES = 8
N = 4096
E = 131072
DIN = 512
D = 256
H = 2
DH = 128
DFF = 2048
EPS = 1e-5
P = 128

NPC = N // N_CORES          # nodes per core = 512
MPC = NPC // P              # m-chunks per core = 4
KT = N // P                 # src k-tiles = 32
KPAD = 32                   # max out-edges per (core, src-node)
KBD = 80                    # max in-edges per dst node
NDUP = 256                  # max duplicate-edge occurrences per core
LPRE = 9                    # attention S/exp prefix depth (kt2 units)
DT16 = mybir.dt.float16
DT32 = mybir.dt.float32
DTI16 = mybir.dt.int16
F = mybir.ActivationFunctionType
A = mybir.AluOpType
INV_SQRT_DH = 1.0 / math.sqrt(DH)


def build_kernel():
    nc = bacc.Bacc("TRN2", target_bir_lowering=False, debug=False,
                   num_devices=N_CORES)

    def din(name, shape, dt=DT32):
        return nc.dram_tensor(name, shape, dt, kind="ExternalInput")

    xT_d = din("xT", [P, MPC * DIN], DT16)
    xTf_d = din("xTf", [P, (DIN // P) * N], DT16)   # full x.T wrapped
    wbdf_d = din("wbdf", [P, (N // P) * KBD], DT16)  # full per-dst weights
    wg_d = din("wg", [P, (DIN // P) * D], DT16)
    warr_d = din("warr", [P, KT * KPAD], DT16)
    idx_d = din("idx", [P, KT * KPAD], DTI16)
    wbd_d = din("wbd", [P, MPC * KBD], DT16)
    dupsr_d = din("dupsr", [P, NDUP // P])
    dupfc_d = din("dupfc", [P, NDUP // P])
    dupw_d = din("dupw", [P, NDUP // P])
    iota1024_d = din("iota1024", [P, KT * KPAD])
    iota128_d = din("iota128", [P, P])
    ident_d = din("ident", [P, P])
    winT_d = din("winT", [P, 2 * 3 * D], DT16)
    ipb_d = din("ipb", [P, 6])
    woT_d = din("woT", [P, 2 * D], DT16)
    w1T_d = din("w1T", [P, 2 * DFF], DT16)
    b1_d = din("b1", [P, DFF // P])
    w2T_d = din("w2T", [P, (DFF // P) * D], DT16)
    bias_d = din("bias", [P, 7 * D])                # host-replicated rows

    out_d = nc.dram_tensor("out", [NPC, D], DT32, kind="ExternalOutput")

    with tile.TileContext(nc) as tc:
        with (
            tc.tile_pool(name="keep", bufs=1) as keep,
            tc.tile_pool(name="dram", bufs=1, space="DRAM") as dram,
        ):
            ones16_col = keep.tile([P, 1], DT16)
            nc.vector.memset(ones16_col[:], 1.0)
            eps_col = keep.tile([P, 1], DT32)
            nc.vector.memset(eps_col[:], EPS)

            lib = nc.gpsimd.load_library(library_config.local_scatter)

            gk = ctx_gcn = tc.tile_pool(name="gcn_keep", bufs=1)
            gk = ctx_gcn.__enter__()

            # ---- A build first: scatters on GpSimd start ASAP ----
            iota1024 = gk.tile([P, KT * KPAD], DT32)
            iota128 = gk.tile([P, P], DT32)
            warr = gk.tile([P, KT * KPAD], DT16)
            idx_t = gk.tile([P, KT * KPAD], DTI16)
            dupsr = gk.tile([P, NDUP // P], DT32)
            dupfc = gk.tile([P, NDUP // P], DT32)
            dupw = gk.tile([P, NDUP // P], DT32)
            nc.sync.dma_start(warr[:], warr_d[:])
            nc.sync.dma_start(idx_t[:], idx_d[:])
            nc.sync.dma_start(iota1024[:], iota1024_d[:])
            nc.sync.dma_start(iota128[:], iota128_d[:])
            nc.sync.dma_start(dupsr[:], dupsr_d[:])
            nc.sync.dma_start(dupfc[:], dupfc_d[:])
            nc.sync.dma_start(dupw[:], dupw_d[:])

            warr16 = gk.tile([P, KT * KPAD], DT16)
            a_tiles = [gk.tile([P, NPC], DT16, tag=f"A{kt}", name=f"A{kt}")
                       for kt in range(KT)]

            scatter_ins = []
            with tc.tile_pool(name="gcn_sb", bufs=2) as gsb, \
                 tc.tile_pool(name="gcn_ps", bufs=2, space="PSUM") as gps:
                mrg_ps = [gps.tile([P, 512], DT32, space="PSUM",
                                   tag=f"mrg{h}", name=f"mrg{h}")
                          for h in range(2)]
                for b in range(NDUP // P):
                    sd = gsb.tile([P, P], DT16, tag="sd")
                    vd = gsb.tile([P, KT * KPAD], DT16, tag="vd")
                    nc.vector.tensor_scalar(sd[:], iota128[:],
                                            dupsr[:, b:b + 1], None,
                                            op0=A.is_equal)
                    nc.vector.tensor_scalar(vd[:], iota1024[:],
                                            dupfc[:, b:b + 1],
                                            dupw[:, b:b + 1],
                                            op0=A.is_equal, op1=A.mult)
                    for h in range(2):
                        nc.tensor.matmul(mrg_ps[h][:], lhsT=sd[:],
                                         rhs=vd[:, 512 * h:512 * h + 512],
                                         start=(b == 0),
                                         stop=(b == NDUP // P - 1))
                for h in range(2):
                    nc.vector.tensor_tensor(warr16[:, 512 * h:512 * h + 512],
                                            warr[:, 512 * h:512 * h + 512],
                                            mrg_ps[h][:], op=A.add)
                for kt in range(KT):
                    ls = nc.gpsimd.local_scatter(
                        a_tiles[kt][:],
                        warr16[:, KPAD * kt:KPAD * (kt + 1)],
                        idx_t[:, KPAD * kt:KPAD * (kt + 1)],
                        channels=P, num_elems=NPC, num_idxs=KPAD,
                    )
                    add_dep_helper(ls.ins, lib.ins, reason="scatter after lib")
                    scatter_ins.append(ls)

            # ---- degrees -> dinv (local + full), rsqrt = exp(-0.5 ln) ----
            wbd = gk.tile([P, MPC * KBD], DT16)
            nc.sync.dma_start(wbd[:], wbd_d[:])
            dinv = gk.tile([P, MPC], DT32)
            dinv2 = gk.tile([P, MPC], DT32)
            deg = gk.tile([P, MPC], DT32)
            nc.vector.tensor_reduce(
                deg[:], wbd[:].rearrange("p (m k) -> p m k", k=KBD),
                axis=mybir.AxisListType.X, op=A.add)
            lgd = gk.tile([P, MPC], DT32)
            nc.scalar.activation(lgd[:], deg[:], F.Ln, bias=1.0, scale=1.0)
            nc.scalar.activation(dinv[:], lgd[:], F.Exp, scale=-0.5)
            nc.vector.tensor_mul(dinv2[:], dinv[:], dinv[:])

            wbdf = gk.tile([P, (N // P) * KBD], DT16)
            nc.sync.dma_start(wbdf[:], wbdf_d[:])
            dinvf = gk.tile([P, N // P], DT32)
            degf = gk.tile([P, N // P], DT32)
            nc.vector.tensor_reduce(
                degf[:], wbdf[:].rearrange("p (j k) -> p j k", k=KBD),
                axis=mybir.AxisListType.X, op=A.add)
            lgdf = gk.tile([P, N // P], DT32)
            nc.scalar.activation(lgdf[:], degf[:], F.Ln, bias=1.0, scale=1.0)
            nc.scalar.activation(dinvf[:], lgdf[:], F.Exp, scale=-0.5)

            # ---- xw = x @ W_gcn: local self-term first, then full ----
            xT16 = gk.tile([P, MPC * DIN], DT16)
            wg16 = gk.tile([P, (DIN // P) * D], DT16)
            nc.sync.dma_start(xT16[:], xT_d[:])
            nc.sync.dma_start(wg16[:], wg_d[:])
            xTf16 = gk.tile([P, (DIN // P) * N], DT16)
            nc.sync.dma_start(xTf16[:], xTf_d[:])
            xws16f = gk.tile([P, (N // P) * D], DT16)
            self32 = gk.tile([P, MPC * D], DT32)
            with tc.tile_pool(name="xw_ps", bufs=4, space="PSUM") as xps:
                for m in range(MPC):
                    pxw = xps.tile([P, D], DT32, space="PSUM", tag="xw")
                    for k in range(DIN // P):
                        nc.tensor.matmul(
                            pxw[:],
                            lhsT=xT16[:, DIN * k + P * m:DIN * k + P * m + P],
                            rhs=wg16[:, D * k:D * (k + 1)],
                            start=(k == 0), stop=(k == DIN // P - 1))
                    nc.vector.tensor_scalar(self32[:, D * m:D * (m + 1)], pxw[:],
                                            dinv2[:, m:m + 1], None, op0=A.mult)
                for j in range(N // P):
                    pxw = xps.tile([P, D], DT32, space="PSUM", tag="xw")
                    for k in range(DIN // P):
                        nc.tensor.matmul(
                            pxw[:],
                            lhsT=xTf16[:, N * k + P * j:N * k + P * (j + 1)],
                            rhs=wg16[:, D * k:D * (k + 1)],
                            start=(k == 0), stop=(k == DIN // P - 1))
                    nc.vector.tensor_scalar(xws16f[:, D * j:D * (j + 1)],
                                            pxw[:], dinvf[:, j:j + 1], None,
                                            op0=A.mult)

            # constants for later phases (DMA after critical ones)
            ident = keep.tile([P, P], DT32)
            ipb = keep.tile([P, 6], DT32)
            b1t = keep.tile([P, DFF // P], DT32)
            nc.sync.dma_start(ident[:], ident_d[:])
            nc.sync.dma_start(ipb[:], ipb_d[:])
            nc.sync.dma_start(b1t[:], b1_d[:])
            winT16 = keep.tile([P, 2 * 3 * D], DT16)
            woT16 = keep.tile([P, 2 * D], DT16)
            nc.sync.dma_start(winT16[:], winT_d[:])
            nc.sync.dma_start(woT16[:], woT_d[:])
            bias_bc = keep.tile([P, 7 * D], DT32)
            nc.sync.dma_start(bias_bc[:], bias_d[:])
            bgcn_bc = bias_bc[:, 0:D]
            b2_bc = bias_bc[:, D:2 * D]
            ln1g_bc = bias_bc[:, 2 * D:3 * D]
            ln1b_bc = bias_bc[:, 3 * D:4 * D]
            ln2g_bc = bias_bc[:, 4 * D:5 * D]
            ln2b_bc = bias_bc[:, 5 * D:6 * D]
            bo_bc = bias_bc[:, 6 * D:7 * D]

            def bc4(ap_2d):
                """[128, D] bias slice -> broadcast [128, MPC, D]."""
                return ap_2d[:, None, :].to_broadcast([P, MPC, D])

            # ---- aggregation: per-kt dependency on its scatter ----
            h_t = keep.tile([P, MPC * D], DT32)
            hT16 = keep.tile([P, 2 * NPC], DT16)
            with tc.tile_pool(name="agg_sb", bufs=1) as asb, \
                 tc.tile_pool(name="agg_ps", bufs=1, space="PSUM") as aps:
                agg_ps = [aps.tile([P, D], DT32, space="PSUM",
                                   tag=f"agg{m}", name=f"agg{m}")
                          for m in range(MPC)]
                for kt in range(KT):
                    for m in range(MPC):
                        agg_mm = nc.tensor.matmul(
                            agg_ps[m][:],
                            lhsT=a_tiles[kt][:, P * m:P * (m + 1)],
                            rhs=xws16f[:, D * kt:D * (kt + 1)],
                            start=(kt == 0), stop=(kt == KT - 1))
                        if m == 0:
                            add_dep_helper(agg_mm.ins, scatter_ins[kt].ins,
                                           reason="agg kt after scatter kt")

                # h = relu(dinv*agg + self + b_gcn)   (batched epilogue)
                x_all = asb.tile([P, MPC * D], DT32, tag="xall")
                for m in range(MPC):
                    nc.vector.scalar_tensor_tensor(
                        x_all[:, D * m:D * (m + 1)], agg_ps[m][:],
                        dinv[:, m:m + 1], self32[:, D * m:D * (m + 1)],
                        op0=A.mult, op1=A.add)
                nc.vector.tensor_tensor(
                    x_all[:].rearrange("p (m d) -> p m d", m=MPC),
                    x_all[:].rearrange("p (m d) -> p m d", m=MPC),
                    bc4(bgcn_bc), op=A.add)
                nc.scalar.activation(h_t[:], x_all[:], F.Relu)

            # transpose h -> hT16 (local feature-major)
            with tc.tile_pool(name="tr_ps", bufs=2, space="PSUM") as tps:
                for m in range(MPC):
                    for f in range(2):
                        ptr = tps.tile([P, P], DT32, space="PSUM", tag="tr")
                        nc.tensor.transpose(
                            ptr[:], h_t[:, D * m + P * f:D * m + P * (f + 1)],
                            ident[:])
                        nc.vector.tensor_copy(
                            hT16[:, NPC * f + P * m:NPC * f + P * (m + 1)],
                            ptr[:])

            ctx_gcn.__exit__(None, None, None)
            ak = ctx_attn = tc.tile_pool(name="attn_keep", bufs=1)
            ak = ctx_attn.__enter__()

            # ---- K^T first, AllGather it; V next, AllGather it; Q last ----
            kT_sb = ak.tile([P, H * NPC], DT16)
            v_sb = ak.tile([P, H * NPC], DT16)
            qT16 = ak.tile([P, H * NPC], DT16)
            with tc.tile_pool(name="kv_ps", bufs=3, space="PSUM") as kvps:
                for h in range(H):
                    pk = kvps.tile([P, NPC], DT32, space="PSUM", tag="kv")
                    for k in range(2):
                        nc.tensor.matmul(
                            pk[:],
                            lhsT=winT16[:, 768 * k + D + P * h:
                                        768 * k + D + P * (h + 1)],
                            rhs=hT16[:, NPC * k:NPC * (k + 1)],
                            start=(k == 0), stop=(k == 1))
                    nc.vector.tensor_scalar(
                        kT_sb[:, NPC * h:NPC * (h + 1)], pk[:],
                        ipb[:, 2 + h:3 + h], None, op0=A.add)

                k_bounce = dram.tile([2 * P, NPC], DT16)
                k_gath = dram.tile([N_CORES * 2 * P, NPC], DT16,
                                   addr_space="Shared")
                nc.scalar.dma_start(
                    k_bounce[:].rearrange("(x p) n -> p x n", p=P),
                    kT_sb[:].rearrange("p (x n) -> p x n", x=2))
                nc.gpsimd.collective_compute(
                    "AllGather", A.bypass,
                    replica_groups=[list(range(N_CORES))],
                    ins=[k_bounce.opt()], outs=[k_gath.opt()])

                for h in range(H):
                    for m in range(MPC):
                        pv = kvps.tile([P, P], DT32, space="PSUM", tag="kvv")
                        for k in range(2):
                            nc.tensor.matmul(
                                pv[:],
                                lhsT=hT16[:, NPC * k + P * m:NPC * k + P * (m + 1)],
                                rhs=winT16[:, 768 * k + 2 * D + P * h:
                                            768 * k + 2 * D + P * (h + 1)],
                                start=(k == 0), stop=(k == 1))
                        nc.vector.tensor_copy(
                            v_sb[:, NPC * h + P * m:NPC * h + P * (m + 1)],
                            pv[:])

                v_bounce = dram.tile([2 * P, NPC], DT16, tag="vb")
                v_gath = dram.tile([N_CORES * 2 * P, NPC], DT16,
                                   addr_space="Shared", tag="vg")
                nc.scalar.dma_start(
                    v_bounce[:].rearrange("(x p) n -> p x n", p=P),
                    v_sb[:].rearrange("p (x n) -> p x n", x=2))
                nc.gpsimd.collective_compute(
                    "AllGather", A.bypass,
                    replica_groups=[list(range(N_CORES))],
                    ins=[v_bounce.opt()], outs=[v_gath.opt()])

                for h in range(H):
                    pq = kvps.tile([P, NPC], DT32, space="PSUM", tag="kv")
                    for k in range(2):
                        nc.tensor.matmul(
                            pq[:],
                            lhsT=winT16[:, 768 * k + P * h:768 * k + P * (h + 1)],
                            rhs=hT16[:, NPC * k:NPC * (k + 1)],
                            start=(k == 0), stop=(k == 1))
                    nc.vector.tensor_scalar(
                        qT16[:, NPC * h:NPC * (h + 1)], pq[:],
                        ipb[:, h:h + 1], None, op0=A.add)

            # FFN weights stream while the AllGathers run
            w1T16 = ak.tile([P, 2 * DFF], DT16)
            nc.sync.dma_start(w1T16[:], w1T_d[:])
            w2T16 = ak.tile([P, (DFF // P) * D], DT16)
            nc.sync.dma_start(w2T16[:], w2T_d[:])

            # residual + out_proj bias, pre-added (Vector idle here)
            hbo = ak.tile([P, MPC * D], DT32)
            nc.vector.tensor_tensor(
                hbo[:].rearrange("p (m d) -> p m d", m=MPC),
                h_t[:].rearrange("p (m d) -> p m d", m=MPC),
                bc4(bo_bc), op=A.add)

            # ---- load gathered K^T / V (simple per-h patterns) ----
            kT16 = ak.tile([P, H * N], DT16)
            v16 = ak.tile([P, H * N], DT16)
            gvk = k_gath[:].rearrange("(g x p) n -> x p g n",
                                      g=N_CORES, x=2, p=P)
            gvv = v_gath[:].rearrange("(g x p) n -> x p g n",
                                      g=N_CORES, x=2, p=P)
            for h in range(H):
                nc.sync.dma_start(
                    kT16[:, N * h:N * (h + 1)].rearrange(
                        "p (g n) -> p g n", g=N_CORES), gvk[h])
            for h in range(H):
                nc.sync.dma_start(
                    v16[:, N * h:N * (h + 1)].rearrange(
                        "p (g n) -> p g n", g=N_CORES), gvv[h])

            # ---- software-pipelined S^T -> exp -> PV + sums ----
            oT16 = ak.tile([P, H * NPC], DT16)
            esum = [ak.tile([P, 2 * NPC], DT16, tag=f"esum{h}",
                            name=f"esum{h}") for h in range(H)]
            KT2 = KT // 2
            with tc.tile_pool(name="att_es", bufs=2 * LPRE, ) as esb, \
                 tc.tile_pool(name="att_sb", bufs=2) as atsb, \
                 tc.tile_pool(name="att_ps", bufs=1, space="PSUM") as atps, \
                 tc.tile_pool(name="s_ps", bufs=2, space="PSUM") as sps:
                o_ps = [atps.tile([P, NPC], DT32, space="PSUM",
                                  tag=f"o{h}", name=f"o{h}")
                        for h in range(H)]
                es_tiles = {}

                def emit_s(kt2):
                    for h in range(H):
                        ps_s = sps.tile([P, 2 * NPC], DT32, space="PSUM",
                                        tag="S")
                        for u in range(2):
                            kt = 2 * kt2 + u
                            nc.tensor.matmul(
                                ps_s[:, NPC * u:NPC * (u + 1)],
                                lhsT=kT16[:, N * h + P * kt:N * h + P * (kt + 1)],
                                rhs=qT16[:, NPC * h:NPC * (h + 1)],
                                start=True, stop=True)
                        es = esb.tile([P, 2 * NPC], DT16, tag="es")
                        nc.scalar.activation(es[:], ps_s[:], F.Exp,
                                             scale=INV_SQRT_DH)
                        es_tiles[(kt2, h)] = es
                        if kt2 == 0:
                            nc.vector.tensor_copy(esum[h][:], es[:])
                        else:
                            nc.vector.tensor_add(esum[h][:], esum[h][:], es[:])

                def emit_pv(kt2):
                    for h in range(H):
                        es = es_tiles.pop((kt2, h))
                        for u in range(2):
                            kt = 2 * kt2 + u
                            nc.tensor.matmul(
                                o_ps[h][:],
                                lhsT=v16[:, N * h + P * kt:N * h + P * (kt + 1)],
                                rhs=es[:, NPC * u:NPC * (u + 1)],
                                start=(kt == 0), stop=(kt == KT - 1))

                for kt2 in range(LPRE):
                    emit_s(kt2)
                for kt2 in range(LPRE, KT2):
                    emit_s(kt2)
                    emit_pv(kt2 - LPRE)

                # denominators: fold esum halves, one matmul per head
                sum_ps = [atps.tile([1, NPC], DT32, space="PSUM",
                                    tag=f"sm{h}", name=f"sm{h}")
                          for h in range(H)]
                for h in range(H):
                    nc.vector.tensor_add(esum[h][:, 0:NPC], esum[h][:, 0:NPC],
                                         esum[h][:, NPC:2 * NPC])
                    nc.tensor.matmul(sum_ps[h][:], lhsT=ones16_col[:],
                                     rhs=esum[h][:, 0:NPC],
                                     start=True, stop=True)

                for kt2 in range(KT2 - LPRE, KT2):
                    emit_pv(kt2)

                # copy unnormalized o to sbuf; transpose sums to
                # per-partition [128, MPC] reciprocals
                recT = atsb.tile([P, H * MPC], DT32, tag="recT", name="recT")
                for h in range(H):
                    nc.vector.tensor_copy(oT16[:, NPC * h:NPC * (h + 1)],
                                          o_ps[h][:])
                    srow = atsb.tile([1, NPC], DT32, tag="srow")
                    nc.vector.tensor_copy(srow[:], sum_ps[h][:])
                    sT_ps = sps.tile([P, MPC], DT32, space="PSUM", tag="S",
                                     name="sTps")
                    for m in range(MPC):
                        nc.tensor.transpose(
                            sT_ps[:, m:m + 1], srow[:, P * m:P * (m + 1)],
                            ident[0:1, 0:1])
                    nc.vector.reciprocal(recT[:, MPC * h:MPC * (h + 1)],
                                         sT_ps[:])

            # ---- o_proj + residual + LN1, pipelined per m-chunk ----
            h1_t = ak.tile([P, MPC * D], DT32)
            h1T16 = ak.tile([P, 2 * NPC], DT16)
            with tc.tile_pool(name="ln_sb", bufs=2) as lsb, \
                 tc.tile_pool(name="op_ps", bufs=2, space="PSUM") as ops, \
                 tc.tile_pool(name="tr2_ps", bufs=2, space="PSUM") as tps2:

                def layernorm_m(dst, x_m, g_sl, b_sl, tag):
                    """LN over feature dim for one [128, D] chunk."""
                    mu = lsb.tile([P, 1], DT32, tag=f"{tag}mu")
                    nc.vector.tensor_reduce(
                        mu[:], x_m.rearrange("p (o d) -> p o d", o=1),
                        axis=mybir.AxisListType.X, op=A.add)
                    negmu = lsb.tile([P, 1], DT32, tag=f"{tag}nm")
                    nc.vector.tensor_scalar(negmu[:], mu[:], -1.0 / D, None,
                                            op0=A.mult)
                    sq = lsb.tile([P, D], DT32, tag=f"{tag}sq")
                    ssq = lsb.tile([P, 1], DT32, tag=f"{tag}ss")
                    nc.scalar.activation(sq[:], x_m, F.Square,
                                         bias=negmu[:], accum_out=ssq[:])
                    lv = lsb.tile([P, 1], DT32, tag=f"{tag}lv")
                    nc.scalar.activation(lv[:], ssq[:], F.Ln, bias=eps_col[:],
                                         scale=1.0 / D)
                    rstd = lsb.tile([P, 1], DT32, tag=f"{tag}rs")
                    nc.scalar.activation(rstd[:], lv[:], F.Exp, scale=-0.5)
                    xc = lsb.tile([P, D], DT32, tag=f"{tag}xc")
                    nc.vector.tensor_scalar(xc[:], x_m, negmu[:], rstd[:],
                                            op0=A.add, op1=A.mult)
                    nc.vector.tensor_tensor(xc[:], xc[:], g_sl, op=A.mult)
                    nc.vector.tensor_tensor(dst, xc[:], b_sl, op=A.add)

                for m in range(MPC):
                    pa = [None, None]
                    for h in range(H):
                        pa[h] = ops.tile([P, D], DT32, space="PSUM", tag="op",
                                         name=f"pa{h}")
                        nc.tensor.matmul(
                            pa[h][:],
                            lhsT=oT16[:, NPC * h + P * m:NPC * h + P * (m + 1)],
                            rhs=woT16[:, D * h:D * (h + 1)],
                            start=True, stop=True)
                    x1m = lsb.tile([P, D], DT32, tag="x1m")
                    nc.vector.tensor_scalar(x1m[:], pa[0][:],
                                            recT[:, m:m + 1], None,
                                            op0=A.mult)
                    nc.vector.scalar_tensor_tensor(
                        x1m[:], pa[1][:], recT[:, MPC + m:MPC + m + 1],
                        x1m[:], op0=A.mult, op1=A.add)
                    nc.vector.tensor_add(x1m[:], x1m[:],
                                         hbo[:, D * m:D * (m + 1)])
                    layernorm_m(h1_t[:, D * m:D * (m + 1)], x1m[:],
                                ln1g_bc, ln1b_bc, "a")
                    for f in range(2):
                        ptr = tps2.tile([P, P], DT32, space="PSUM", tag="tr2")
                        nc.tensor.transpose(
                            ptr[:],
                            h1_t[:, D * m + P * f:D * m + P * (f + 1)],
                            ident[:])
                        nc.vector.tensor_copy(
                            h1T16[:, NPC * f + P * m:NPC * f + P * (m + 1)],
                            ptr[:])

                # ---- FFN ----
                ff1T = ak.tile([P, (DFF // P) * NPC], DT16)
                with tc.tile_pool(name="f1_ps", bufs=3, space="PSUM") as fps:
                    for dc in range(DFF // P):
                        pf = fps.tile([P, NPC], DT32, space="PSUM", tag="f1")
                        for k in range(2):
                            nc.tensor.matmul(
                                pf[:],
                                lhsT=w1T16[:, DFF * k + P * dc:
                                           DFF * k + P * (dc + 1)],
                                rhs=h1T16[:, NPC * k:NPC * (k + 1)],
                                start=(k == 0), stop=(k == 1))
                        nc.scalar.activation(
                            ff1T[:, NPC * dc:NPC * (dc + 1)], pf[:], F.Relu,
                            bias=b1t[:, dc:dc + 1])

                with tc.tile_pool(name="f2_ps", bufs=2, space="PSUM") as fps2:
                    for m in range(MPC):
                        pf2 = fps2.tile([P, D], DT32, space="PSUM", tag="f2")
                        for kt2 in range(DFF // P):
                            nc.tensor.matmul(
                                pf2[:],
                                lhsT=ff1T[:, NPC * kt2 + P * m:
                                          NPC * kt2 + P * (m + 1)],
                                rhs=w2T16[:, D * kt2:D * (kt2 + 1)],
                                start=(kt2 == 0), stop=(kt2 == DFF // P - 1))
                        x2m = lsb.tile([P, D], DT32, tag="x2m")
                        nc.vector.scalar_tensor_tensor(
                            x2m[:], pf2[:], 1.0,
                            h1_t[:, D * m:D * (m + 1)], op0=A.mult, op1=A.add)
                        nc.vector.tensor_tensor(x2m[:], x2m[:], b2_bc,
                                                op=A.add)
                        out_m = lsb.tile([P, D], DT32, tag="outm")
                        layernorm_m(out_m[:], x2m[:], ln2g_bc, ln2b_bc, "b")
                        nc.scalar.dma_start(
                            out_d[:].rearrange("(m p) d -> m p d", p=P)[m],
                            out_m[:])
            ctx_attn.__exit__(None, None, None)

    nc.compile()
    return nc


# ======================= host-side prep =======================

def _prep_inputs(x, edge_index, edge_weight, W_gcn, b_gcn, in_proj_w,
                 in_proj_b, out_proj_w, out_proj_b, lin1_w, lin1_b, lin2_w,
                 lin2_b, ln1_g, ln1_b, ln2_g, ln2_b):
    """Pure index-permutation / layout prep. Returns per-core input maps."""
    x = np.asarray(x, np.float32)
    src = np.asarray(edge_index[0], np.int64)
    dst = np.asarray(edge_index[1], np.int64)
    w = np.asarray(edge_weight, np.float32)

    def wrap128(a):
        # [n*128, m] -> [128, n*m] with col block t <- rows [128t, 128t+128)
        n = a.shape[0] // P
        return np.ascontiguousarray(
            a.reshape(n, P, a.shape[1]).transpose(1, 0, 2).reshape(P, -1))

    iota1024 = np.tile(np.arange(KT * KPAD, dtype=np.float32), (P, 1))
    iota128 = np.tile(np.arange(P, dtype=np.float32), (P, 1))
    ident = np.eye(P, dtype=np.float32)
    # out_proj bias with the V-bias term folded in (softmax rows sum to 1)
    bv = np.asarray(in_proj_b, np.float32)[2 * D:3 * D]
    bo_eff = (np.asarray(out_proj_b, np.float32)
              + bv @ np.asarray(out_proj_w, np.float32).T)
    bias_row = np.concatenate([
        np.asarray(v, np.float32).reshape(-1) for v in
        (b_gcn, lin2_b, ln1_g, ln1_b, ln2_g, ln2_b, bo_eff)
    ]).reshape(1, -1)
    bias_stack = np.ascontiguousarray(np.tile(bias_row, (P, 1)))

    f16 = np.float16
    shared = {
        "wg": wrap128(np.asarray(W_gcn, np.float32)).astype(f16),
        "iota1024": iota1024, "iota128": iota128,
        "ident": ident,
        "winT": wrap128(np.ascontiguousarray(
            np.asarray(in_proj_w, np.float32).T)).astype(f16),
        "ipb": np.ascontiguousarray(
            np.asarray(in_proj_b, np.float32).reshape(6, P).T),
        "woT": wrap128(np.ascontiguousarray(
            np.asarray(out_proj_w, np.float32).T)).astype(f16),
        "w1T": wrap128(np.ascontiguousarray(
            np.asarray(lin1_w, np.float32).T)).astype(f16),
        "b1": np.ascontiguousarray(
            np.asarray(lin1_b, np.float32).reshape(DFF // P, P).T),
        "w2T": wrap128(np.ascontiguousarray(
            np.asarray(lin2_w, np.float32).T)).astype(f16),
        "bias": bias_stack,
    }

    shared_xTf = wrap128(np.ascontiguousarray(x.T)).astype(f16)
    # full per-dst weight table for replicated degree computation
    wbdf = np.zeros((N, KBD), np.float32)
    cntf = np.zeros(N, np.int32)
    for di, wi in zip(dst.tolist(), w.tolist()):
        j = int(cntf[di])
        assert j < KBD
        wbdf[di, j] = wi
        cntf[di] = j + 1
    wbdf_full_w = wrap128(wbdf).astype(f16)

    core_of = dst // NPC
    in_maps = []
    for c in range(N_CORES):
        sel = np.nonzero(core_of == c)[0]
        s_c = src[sel]
        d_c = (dst[sel] - NPC * c).astype(np.int64)
        w_c = w[sel]

        w_arr = np.zeros((N, KPAD), np.float32)
        idx_arr = np.full((N, KPAD), -1, np.int16)
        counts = np.zeros(N, np.int32)
        first_slot = {}
        dup_sr, dup_fc, dup_w = [], [], []
        for si, di, wi in zip(s_c.tolist(), d_c.tolist(), w_c.tolist()):
            key = si * NPC + di
            slot = first_slot.get(key)
            if slot is None:
                j = int(counts[si])
                assert j < KPAD, f"KPAD overflow at src {si}"
                counts[si] = j + 1
                w_arr[si, j] = wi
                idx_arr[si, j] = di
                first_slot[key] = j
            else:
                dup_sr.append(si % P)
                dup_fc.append(KPAD * (si // P) + slot)
                dup_w.append(wi)
        assert len(dup_sr) <= NDUP, f"NDUP overflow: {len(dup_sr)}"

        def pad_dup(vals, dtype=np.float32):
            a = np.zeros(NDUP, dtype)
            a[:len(vals)] = vals
            return np.ascontiguousarray(a.reshape(NDUP // P, P).T)

        wbd = np.zeros((NPC, KBD), np.float32)
        cnt2 = np.zeros(NPC, np.int32)
        for di, wi in zip(d_c.tolist(), w_c.tolist()):
            j = int(cnt2[di])
            assert j < KBD, f"KBD overflow at dst {di}"
            wbd[di, j] = wi
            cnt2[di] = j + 1

        in_maps.append({
            **shared,
            "xT": wrap128(np.ascontiguousarray(
                x[NPC * c:NPC * (c + 1)].T)).astype(f16),
            "xTf": shared_xTf,
            "wbdf": wbdf_full_w,
            "warr": wrap128(w_arr).astype(f16),
            "idx": wrap128(idx_arr),
            "wbd": wrap128(wbd).astype(f16),
            "dupsr": pad_dup(dup_sr),
            "dupfc": pad_dup(dup_fc),
            "dupw": pad_dup(dup_w),
        })
    return in_maps


# ======================= runner =======================

class _Runner:
    """Persistent-jit SPMD executor (mirrors bass2jax.run_bass_via_pjrt)."""

    def __init__(self, nc):
        import jax
        from jax.sharding import Mesh, PartitionSpec
        from jax.experimental.shard_map import shard_map
        from concourse.bass2jax import (_bass_exec_p, install_neuronx_cc_hook,
                                        partition_id_tensor)
        install_neuronx_cc_hook()
        self.jax = jax
        partition_name = (nc.partition_id_tensor.name
                          if nc.partition_id_tensor else None)
        in_names, out_names, out_avals, zero_outs = [], [], [], []
        for alloc in nc.m.functions[0].allocations:
            if not isinstance(alloc, mybir.MemoryLocationSet):
                continue
            name = alloc.memorylocations[0].name
            if alloc.kind == "ExternalInput":
                if name != partition_name:
                    in_names.append(name)
            elif alloc.kind == "ExternalOutput":
                out_names.append(name)
                shape = tuple(alloc.tensor_shape)
                dtype = mybir.dt.np(alloc.dtype)
                out_avals.append(jax.core.ShapedArray(shape, dtype))
                zero_outs.append(np.zeros(shape, dtype))
        self.in_names, self.out_names = in_names, out_names
        self.out_shapes = [tuple(a.shape) for a in out_avals]
        self.n_params = len(in_names)
        self.zero_outs = zero_outs
        all_in = in_names + out_names
        if partition_name is not None:
            all_in.append(partition_name)

        def _body(*args):
            operands = list(args)
            if partition_name is not None:
                operands.append(partition_id_tensor())
            return tuple(_bass_exec_p.bind(
                *operands, out_avals=tuple(out_avals), in_names=tuple(all_in),
                out_names=tuple(out_names), lowering_input_output_aliases=(),
                sim_require_finite=True, sim_require_nnan=True, nc=nc))

        devices = jax.devices()[:N_CORES]
        self.mesh = Mesh(np.asarray(devices), ("core",))
        nin = self.n_params + len(out_names)
        self.fn = jax.jit(
            shard_map(_body, mesh=self.mesh,
                      in_specs=(PartitionSpec("core"),) * nin,
                      out_specs=(PartitionSpec("core"),) * len(out_names),
                      check_rep=False),
            keep_unused=True)

    def place(self, in_maps):
        import jax
        from jax.sharding import PartitionSpec
        per_core = [[np.asarray(m[n]) for n in self.in_names] for m in in_maps]
        concat = [np.concatenate([per_core[c][i] for c in range(N_CORES)], axis=0)
                  for i in range(self.n_params)]
        zeros = [np.zeros((N_CORES * z.shape[0], *z.shape[1:]), z.dtype)
                 for z in self.zero_outs]
        sh = jax.sharding.NamedSharding(self.mesh, PartitionSpec("core"))
        return [jax.device_put(a, sh) for a in (*concat, *zeros)]

    def run(self, args):
        outs = self.fn(*args)
        self.jax.block_until_ready(outs)
        return outs

    def results(self, outs):
        res = []
        for c in range(N_CORES):
            d = {}
            for i, name in enumerate(self.out_names):
                full = np.asarray(outs[i])
                ps = self.out_shapes[i]
                d[name] = full.reshape((N_CORES,) + ps)[c]
            res.append(d)
        return res


_CACHE = {}


def _get_runner():
    if "runner" not in _CACHE:
        nc = build_kernel()
        _CACHE["nc"] = nc
        _CACHE["runner"] = _Runner(nc)
    return _CACHE["runner"]


def kernel(**inputs) -> np.ndarray:
    runner = _get_runner()
    in_maps = _prep_inputs(**inputs)
    args = runner.place(in_maps)
    outs = runner.run(args)
    res = runner.results(outs)
    return np.concatenate([res[c]["out"] for c in range(N_CORES)], axis=0)


# revision 21
# speedup vs baseline: 1.2372x; 1.1970x over previous
"""GCNEncoder (GCNConv + TransformerEncoderLayer) on 8 Trainium2 NeuronCores.

Sharding: nodes are split 512/core (8 cores). Per core:
  - GCN: dense normalized-adjacency block A^T [4096 src, 512 dst] built on
    device via GPSIMD local_scatter from host-permuted (index-only) edge
    layouts; aggregation is a dense fp16 matmul against replicated scaled
    features, pipelined per src k-tile against scatter completion.
  - Attention: both heads, q = the core's 512 nodes vs all 4096 keys.
    K^T is AllGathered first and S/exp stream against it while the V
    AllGather is still in flight; PV matmuls join the stream once V lands
    (software-pipelined schedule, in-order PE friendly). Softmax skips
    max-subtraction; denominators via one ones-matmul per head over a
    Vector-accumulated exp-sum.
  - FFN + both LayerNorms local, LN pipelined per 128-node chunk.
All rsqrt computed as exp(-0.5*ln(x)) so one activation table serves the
whole kernel. All matmul operands fp16, accumulation fp32 in PSUM.
"""

import math

import numpy as np

import concourse.bacc as bacc
import concourse.mybir as mybir
import concourse.tile as tile
from concourse import library_config
from concourse.tile_rust import add_dep_helper

N_CORES = 8
N = 4096
E = 131072
DIN = 512
D = 256
H = 2
DH = 128
DFF = 2048
EPS = 1e-5
P = 128

NPC = N // N_CORES          # nodes per core = 512
MPC = NPC // P              # m-chunks per core = 4
KT = N // P                 # src k-tiles = 32
KPAD = 32                   # max out-edges per (core, src-node)
KBD = 80                    # max in-edges per dst node
NDUP = 256                  # max duplicate-edge occurrences per core
LPRE = 11                   # attention S/exp prefix depth (kt2 units)
DT16 = mybir.dt.float16
DT32 = mybir.dt.float32
DTI16 = mybir.dt.int16
F = mybir.ActivationFunctionType
A = mybir.AluOpType
INV_SQRT_DH = 1.0 / math.sqrt(DH)


def build_kernel():
    nc = bacc.Bacc("TRN2", target_bir_lowering=False, debug=False,
                   num_devices=N_CORES)

    def din(name, shape, dt=DT32):
        return nc.dram_tensor(name, shape, dt, kind="ExternalInput")

    xT_d = din("xT", [P, MPC * DIN], DT16)
    xTf_d = din("xTf", [P, KT * MPC * P], DT16)     # full x.T, j-major wrap
    wbdf_d = din("wbdf", [P, (N // P) * KBD], DT16)  # full per-dst weights
    wg_d = din("wg", [P, (DIN // P) * D], DT16)
    warr_d = din("warr", [P, KT * KPAD], DT16)
    idx_d = din("idx", [P, KT * KPAD], DTI16)
    wbd_d = din("wbd", [P, MPC * KBD], DT16)
    ident_d = din("ident", [P, P])
    winT_d = din("winT", [P, 2 * 3 * D], DT16)
    ipb_d = din("ipb", [P, 6])
    woT_d = din("woT", [P, 2 * D], DT16)
    w1T_d = din("w1T", [P, 2 * DFF], DT16)
    b1_d = din("b1", [P, DFF // P])
    w2T_d = din("w2T", [P, (DFF // P) * D], DT16)
    bias_d = din("bias", [P, 7 * D])                # host-replicated rows

    out_d = nc.dram_tensor("out", [NPC, D], DT32, kind="ExternalOutput")

    with tile.TileContext(nc) as tc:
        with (
            tc.tile_pool(name="keep", bufs=1) as keep,
            tc.tile_pool(name="dram", bufs=1, space="DRAM") as dram,
        ):
            ones16_col = keep.tile([P, 1], DT16)
            nc.vector.memset(ones16_col[:], 1.0)

            lib = nc.gpsimd.load_library(library_config.local_scatter)

            gk = ctx_gcn = tc.tile_pool(name="gcn_keep", bufs=1)
            gk = ctx_gcn.__enter__()

            # warmup collective: absorbs the CC engine's first-program setup
            # cost long before the gathers that matter
            cc_warm = keep.tile([P, 2], DT16)
            nc.vector.memset(cc_warm[:], 0.0)
            warm_b = dram.tile([P, 2], DT16)
            warm_g = dram.tile([N_CORES * P, 2], DT16, addr_space="Shared")
            nc.scalar.dma_start(warm_b[:], cc_warm[:])
            nc.gpsimd.collective_compute(
                "AllGather", A.bypass,
                replica_groups=[list(range(N_CORES))],
                ins=[warm_b.opt()], outs=[warm_g.opt()])

            # ---- A build first: scatters on GpSimd start ASAP ----
            warr = gk.tile([P, KT * KPAD], DT16)
            idx_t = gk.tile([P, KT * KPAD], DTI16)
            nc.sync.dma_start(warr[:], warr_d[:])
            nc.sync.dma_start(idx_t[:], idx_d[:])

            a_tiles = [gk.tile([P, NPC], DT16, tag=f"A{kt}", name=f"A{kt}")
                       for kt in range(KT)]
            scatter_ins = []
            for kt in range(KT):
                ls = nc.gpsimd.local_scatter(
                    a_tiles[kt][:],
                    warr[:, KPAD * kt:KPAD * (kt + 1)],
                    idx_t[:, KPAD * kt:KPAD * (kt + 1)],
                    channels=P, num_elems=NPC, num_idxs=KPAD,
                )
                add_dep_helper(ls.ins, lib.ins, reason="scatter after lib")
                scatter_ins.append(ls)

            # ---- degrees -> dinv (local + full), rsqrt = exp(-0.5 ln) ----
            wbd = gk.tile([P, MPC * KBD], DT16)
            nc.sync.dma_start(wbd[:], wbd_d[:])
            dinv = gk.tile([P, MPC], DT32)
            dinv2 = gk.tile([P, MPC], DT32)
            deg = gk.tile([P, MPC], DT32)
            nc.vector.tensor_reduce(
                deg[:], wbd[:].rearrange("p (m k) -> p m k", k=KBD),
                axis=mybir.AxisListType.X, op=A.add)
            sqd = gk.tile([P, MPC], DT32)
            nc.scalar.activation(sqd[:], deg[:], F.Sqrt, bias=1.0, scale=1.0)
            nc.vector.reciprocal(dinv[:], sqd[:])
            nc.vector.tensor_mul(dinv2[:], dinv[:], dinv[:])

            wbdf = gk.tile([P, (N // P) * KBD], DT16)
            nc.sync.dma_start(wbdf[:], wbdf_d[:])
            dinvf = gk.tile([P, N // P], DT32)
            degf = gk.tile([P, N // P], DT32)
            nc.vector.tensor_reduce(
                degf[:], wbdf[:].rearrange("p (j k) -> p j k", k=KBD),
                axis=mybir.AxisListType.X, op=A.add)
            sqdf = gk.tile([P, N // P], DT32)
            nc.scalar.activation(sqdf[:], degf[:], F.Sqrt, bias=1.0, scale=1.0)
            nc.vector.reciprocal(dinvf[:], sqdf[:])

            # ---- xw = x @ W_gcn: local self-term first, then full ----
            xT16 = gk.tile([P, MPC * DIN], DT16)
            wg16 = gk.tile([P, (DIN // P) * D], DT16)
            nc.sync.dma_start(xT16[:], xT_d[:])
            nc.sync.dma_start(wg16[:], wg_d[:])
            xTf16 = gk.tile([P, (DIN // P) * N], DT16)
            nc.sync.dma_start(xTf16[:], xTf_d[:])
            xws16f = gk.tile([P, (N // P) * D], DT16)
            self32 = gk.tile([P, MPC * D], DT32)
            with tc.tile_pool(name="xw_ps", bufs=4, space="PSUM") as xps:
                for m in range(MPC):
                    pxw = xps.tile([P, D], DT32, space="PSUM", tag="xw")
                    for k in range(DIN // P):
                        nc.tensor.matmul(
                            pxw[:],
                            lhsT=xT16[:, DIN * k + P * m:DIN * k + P * m + P],
                            rhs=wg16[:, D * k:D * (k + 1)],
                            start=(k == 0), stop=(k == DIN // P - 1))
                    nc.vector.tensor_scalar(self32[:, D * m:D * (m + 1)], pxw[:],
                                            dinv2[:, m:m + 1], None, op0=A.mult)
                for j in range(N // P):
                    pxw = xps.tile([P, D], DT32, space="PSUM", tag="xw")
                    for k in range(DIN // P):
                        nc.tensor.matmul(
                            pxw[:],
                            lhsT=xTf16[:, MPC * P * j + P * k:
                                       MPC * P * j + P * (k + 1)],
                            rhs=wg16[:, D * k:D * (k + 1)],
                            start=(k == 0), stop=(k == DIN // P - 1))
                    nc.vector.tensor_scalar(xws16f[:, D * j:D * (j + 1)],
                                            pxw[:], dinvf[:, j:j + 1], None,
                                            op0=A.mult)

            # constants for later phases (DMA after critical ones)
            ident = keep.tile([P, P], DT32)
            ipb = keep.tile([P, 6], DT32)
            b1t = keep.tile([P, DFF // P], DT32)
            nc.sync.dma_start(ident[:], ident_d[:])
            nc.sync.dma_start(ipb[:], ipb_d[:])
            nc.sync.dma_start(b1t[:], b1_d[:])
            winT16 = keep.tile([P, 2 * 3 * D], DT16)
            woT16 = keep.tile([P, 2 * D], DT16)
            nc.sync.dma_start(winT16[:], winT_d[:])
            nc.sync.dma_start(woT16[:], woT_d[:])
            bias_bc = keep.tile([P, 7 * D], DT32)
            nc.sync.dma_start(bias_bc[:], bias_d[:])
            bgcn_bc = bias_bc[:, 0:D]
            b2_bc = bias_bc[:, D:2 * D]
            ln1g_bc = bias_bc[:, 2 * D:3 * D]
            ln1b_bc = bias_bc[:, 3 * D:4 * D]
            ln2g_bc = bias_bc[:, 4 * D:5 * D]
            ln2b_bc = bias_bc[:, 5 * D:6 * D]
            bo_bc = bias_bc[:, 6 * D:7 * D]

            def bc4(ap_2d):
                """[128, D] bias slice -> broadcast [128, MPC, D]."""
                return ap_2d[:, None, :].to_broadcast([P, MPC, D])

            # ---- aggregation: per-kt dependency on its scatter ----
            h_t = keep.tile([P, MPC * D], DT32)
            hT16 = keep.tile([P, 2 * NPC], DT16)
            with tc.tile_pool(name="agg_sb", bufs=2) as asb, \
                 tc.tile_pool(name="agg_ps", bufs=1, space="PSUM") as aps:
                agg_ps = [aps.tile([P, D], DT32, space="PSUM",
                                   tag=f"agg{m}", name=f"agg{m}")
                          for m in range(MPC)]
                for kt in range(KT):
                    for m in range(MPC):
                        agg_mm = nc.tensor.matmul(
                            agg_ps[m][:],
                            lhsT=a_tiles[kt][:, P * m:P * (m + 1)],
                            rhs=xws16f[:, D * kt:D * (kt + 1)],
                            start=(kt == 0), stop=(kt == KT - 1))
                        if m == 0:
                            add_dep_helper(agg_mm.ins, scatter_ins[kt].ins,
                                           reason="agg kt after scatter kt")

                # h = relu(dinv*agg + self + b_gcn), per-m so transposes start
                # as soon as the first chunk is through the epilogue
                with tc.tile_pool(name="tr_ps", bufs=2, space="PSUM") as tps:
                    for m in range(MPC):
                        x_m = asb.tile([P, D], DT32, tag="xm")
                        nc.vector.scalar_tensor_tensor(
                            x_m[:], agg_ps[m][:],
                            dinv[:, m:m + 1], self32[:, D * m:D * (m + 1)],
                            op0=A.mult, op1=A.add)
                        nc.vector.tensor_tensor(x_m[:], x_m[:], bgcn_bc,
                                                op=A.add)
                        nc.scalar.activation(h_t[:, D * m:D * (m + 1)],
                                             x_m[:], F.Relu)
                        for f in range(2):
                            ptr = tps.tile([P, P], DT32, space="PSUM",
                                           tag="tr")
                            nc.tensor.transpose(
                                ptr[:],
                                h_t[:, D * m + P * f:D * m + P * (f + 1)],
                                ident[:])
                            nc.vector.tensor_copy(
                                hT16[:, NPC * f + P * m:NPC * f + P * (m + 1)],
                                ptr[:])
            # prefetch the exp activation table during the collective window
            dummy_e = keep.tile([P, 1], DT32)
            nc.scalar.activation(dummy_e[:], dinv[:, 0:1], F.Exp)

            ctx_gcn.__exit__(None, None, None)
            ak = ctx_attn = tc.tile_pool(name="attn_keep", bufs=1)
            ak = ctx_attn.__enter__()

            # ---- K^T first, AllGather it; V next, AllGather it; Q last ----
            kT_sb = ak.tile([P, H * NPC], DT16)
            v_sb = ak.tile([P, H * NPC], DT16)
            qT16 = ak.tile([P, H * NPC], DT16)
            with tc.tile_pool(name="kv_ps", bufs=3, space="PSUM") as kvps:
                for h in range(H):
                    pk = kvps.tile([P, NPC], DT32, space="PSUM", tag="kv")
                    for k in range(2):
                        nc.tensor.matmul(
                            pk[:],
                            lhsT=winT16[:, 768 * k + D + P * h:
                                        768 * k + D + P * (h + 1)],
                            rhs=hT16[:, NPC * k:NPC * (k + 1)],
                            start=(k == 0), stop=(k == 1))
                    nc.vector.tensor_scalar(
                        kT_sb[:, NPC * h:NPC * (h + 1)], pk[:],
                        ipb[:, 2 + h:3 + h], None, op0=A.add)

                k_bounce = dram.tile([2 * P, NPC], DT16)
                k_gath = dram.tile([N_CORES * 2 * P, NPC], DT16,
                                   addr_space="Shared")
                nc.scalar.dma_start(
                    k_bounce[:].rearrange("(x p) n -> p x n", p=P),
                    kT_sb[:].rearrange("p (x n) -> p x n", x=2))
                nc.gpsimd.collective_compute(
                    "AllGather", A.bypass,
                    replica_groups=[list(range(N_CORES))],
                    ins=[k_bounce.opt()], outs=[k_gath.opt()])

                for h in range(H):
                    for m in range(MPC):
                        pv = kvps.tile([P, P], DT32, space="PSUM", tag="kvv")
                        for k in range(2):
                            nc.tensor.matmul(
                                pv[:],
                                lhsT=hT16[:, NPC * k + P * m:NPC * k + P * (m + 1)],
                                rhs=winT16[:, 768 * k + 2 * D + P * h:
                                            768 * k + 2 * D + P * (h + 1)],
                                start=(k == 0), stop=(k == 1))
                        nc.vector.tensor_copy(
                            v_sb[:, NPC * h + P * m:NPC * h + P * (m + 1)],
                            pv[:])

                v_bounce = dram.tile([2 * P, NPC], DT16, tag="vb")
                v_gath = dram.tile([N_CORES * 2 * P, NPC], DT16,
                                   addr_space="Shared", tag="vg")
                nc.scalar.dma_start(
                    v_bounce[:].rearrange("(x p) n -> p x n", p=P),
                    v_sb[:].rearrange("p (x n) -> p x n", x=2))
                nc.gpsimd.collective_compute(
                    "AllGather", A.bypass,
                    replica_groups=[list(range(N_CORES))],
                    ins=[v_bounce.opt()], outs=[v_gath.opt()])

                for h in range(H):
                    pq = kvps.tile([P, NPC], DT32, space="PSUM", tag="kv")
                    for k in range(2):
                        nc.tensor.matmul(
                            pq[:],
                            lhsT=winT16[:, 768 * k + P * h:768 * k + P * (h + 1)],
                            rhs=hT16[:, NPC * k:NPC * (k + 1)],
                            start=(k == 0), stop=(k == 1))
                    nc.vector.tensor_scalar(
                        qT16[:, NPC * h:NPC * (h + 1)], pq[:],
                        ipb[:, h:h + 1], None, op0=A.add)

            # FFN weights stream while the AllGathers run
            w1T16 = ak.tile([P, 2 * DFF], DT16)
            nc.sync.dma_start(w1T16[:], w1T_d[:])
            w2T16 = ak.tile([P, (DFF // P) * D], DT16)
            nc.sync.dma_start(w2T16[:], w2T_d[:])

            # residual + out_proj bias, pre-added (Vector idle here)
            hbo = ak.tile([P, MPC * D], DT32)
            nc.vector.tensor_tensor(
                hbo[:].rearrange("p (m d) -> p m d", m=MPC),
                h_t[:].rearrange("p (m d) -> p m d", m=MPC),
                bc4(bo_bc), op=A.add)

            # ---- load gathered K^T / V (per-core simple blocks so the
            # first S / PV matmuls start on block 0 while later blocks
            # are still in flight) ----
            kT16 = ak.tile([P, H * N], DT16)
            v16 = ak.tile([P, H * N], DT16)
            gvk = k_gath[:].rearrange("(g x p) n -> g x p n",
                                      g=N_CORES, x=2, p=P)
            gvv = v_gath[:].rearrange("(g x p) n -> g x p n",
                                      g=N_CORES, x=2, p=P)
            for g in range(N_CORES):
                for h in range(H):
                    nc.sync.dma_start(
                        kT16[:, N * h + NPC * g:N * h + NPC * (g + 1)],
                        gvk[g, h])
            for g in range(N_CORES):
                for h in range(H):
                    nc.sync.dma_start(
                        v16[:, N * h + NPC * g:N * h + NPC * (g + 1)],
                        gvv[g, h])

            # ---- software-pipelined S^T -> exp -> PV + sums ----
            oT16 = ak.tile([P, H * NPC], DT16)
            esum = [ak.tile([P, 2 * NPC], DT16, tag=f"esum{h}",
                            name=f"esum{h}") for h in range(H)]
            KT2 = KT // 2
            with tc.tile_pool(name="att_es", bufs=2 * LPRE + 4) as esb, \
                 tc.tile_pool(name="att_sb", bufs=2) as atsb, \
                 tc.tile_pool(name="att_ps", bufs=1, space="PSUM") as atps, \
                 tc.tile_pool(name="s_ps", bufs=2, space="PSUM") as sps:
                o_ps = [atps.tile([P, NPC], DT32, space="PSUM",
                                  tag=f"o{h}", name=f"o{h}")
                        for h in range(H)]
                es_tiles = {}

                def emit_s(kt2):
                    for h in range(H):
                        ps_s = sps.tile([P, 2 * NPC], DT32, space="PSUM",
                                        tag="S")
                        for u in range(2):
                            kt = 2 * kt2 + u
                            nc.tensor.matmul(
                                ps_s[:, NPC * u:NPC * (u + 1)],
                                lhsT=kT16[:, N * h + P * kt:N * h + P * (kt + 1)],
                                rhs=qT16[:, NPC * h:NPC * (h + 1)],
                                start=True, stop=True)
                        es = esb.tile([P, 2 * NPC], DT16, tag="es")
                        nc.scalar.activation(es[:], ps_s[:], F.Exp,
                                             scale=INV_SQRT_DH)
                        es_tiles[(kt2, h)] = es
                        if kt2 == 0:
                            nc.vector.tensor_copy(esum[h][:], es[:])
                        else:
                            nc.vector.tensor_add(esum[h][:], esum[h][:], es[:])

                def emit_pv(kt2):
                    for h in range(H):
                        es = es_tiles.pop((kt2, h))
                        for u in range(2):
                            kt = 2 * kt2 + u
                            nc.tensor.matmul(
                                o_ps[h][:],
                                lhsT=v16[:, N * h + P * kt:N * h + P * (kt + 1)],
                                rhs=es[:, NPC * u:NPC * (u + 1)],
                                start=(kt == 0), stop=(kt == KT - 1))

                for kt2 in range(LPRE):
                    emit_s(kt2)
                for kt2 in range(LPRE, KT2):
                    emit_s(kt2)
                    emit_pv(kt2 - LPRE)

                # denominators: fold esum halves, one matmul per head
                sum_ps = [atps.tile([1, NPC], DT32, space="PSUM",
                                    tag=f"sm{h}", name=f"sm{h}")
                          for h in range(H)]
                for h in range(H):
                    nc.vector.tensor_add(esum[h][:, 0:NPC], esum[h][:, 0:NPC],
                                         esum[h][:, NPC:2 * NPC])
                    nc.tensor.matmul(sum_ps[h][:], lhsT=ones16_col[:],
                                     rhs=esum[h][:, 0:NPC],
                                     start=True, stop=True)

                for kt2 in range(KT2 - LPRE, KT2):
                    emit_pv(kt2)

                # copy unnormalized o to sbuf; transpose sums to
                # per-partition [128, MPC] reciprocals
                recT = atsb.tile([P, H * MPC], DT32, tag="recT", name="recT")
                for h in range(H):
                    nc.vector.tensor_copy(oT16[:, NPC * h:NPC * (h + 1)],
                                          o_ps[h][:])
                    srow = atsb.tile([1, NPC], DT32, tag="srow")
                    nc.vector.tensor_copy(srow[:], sum_ps[h][:])
                    sT_ps = sps.tile([P, MPC], DT32, space="PSUM", tag="S",
                                     name="sTps")
                    for m in range(MPC):
                        nc.tensor.transpose(
                            sT_ps[:, m:m + 1], srow[:, P * m:P * (m + 1)],
                            ident[0:1, 0:1])
                    nc.vector.reciprocal(recT[:, MPC * h:MPC * (h + 1)],
                                         sT_ps[:])

            # ---- o_proj + residual + LN1, pipelined per m-chunk ----
            h1_t = ak.tile([P, MPC * D], DT32)
            h1T16 = ak.tile([P, 2 * NPC], DT16)
            with tc.tile_pool(name="ln_sb", bufs=2) as lsb, \
                 tc.tile_pool(name="op_ps", bufs=2, space="PSUM") as ops, \
                 tc.tile_pool(name="tr2_ps", bufs=2, space="PSUM") as tps2:

                def layernorm_m(dst, x_m, g_sl, b_sl, tag):
                    """LN over feature dim for one [128, D] chunk."""
                    mu = lsb.tile([P, 1], DT32, tag=f"{tag}mu")
                    nc.vector.tensor_reduce(
                        mu[:], x_m.rearrange("p (o d) -> p o d", o=1),
                        axis=mybir.AxisListType.X, op=A.add)
                    negmu = lsb.tile([P, 1], DT32, tag=f"{tag}nm")
                    nc.vector.tensor_scalar(negmu[:], mu[:], -1.0 / D, None,
                                            op0=A.mult)
                    sq = lsb.tile([P, D], DT32, tag=f"{tag}sq")
                    ssq = lsb.tile([P, 1], DT32, tag=f"{tag}ss")
                    nc.scalar.activation(sq[:], x_m, F.Square,
                                         bias=negmu[:], accum_out=ssq[:])
                    var = lsb.tile([P, 1], DT32, tag=f"{tag}vr")
                    nc.vector.tensor_scalar(var[:], ssq[:], 1.0 / D, EPS,
                                            op0=A.mult, op1=A.add)
                    sd = lsb.tile([P, 1], DT32, tag=f"{tag}sd")
                    nc.scalar.activation(sd[:], var[:], F.Sqrt)
                    rstd = lsb.tile([P, 1], DT32, tag=f"{tag}rs")
                    nc.vector.reciprocal(rstd[:], sd[:])
                    xc = lsb.tile([P, D], DT32, tag=f"{tag}xc")
                    nc.vector.tensor_scalar(xc[:], x_m, negmu[:], rstd[:],
                                            op0=A.add, op1=A.mult)
                    nc.vector.tensor_tensor(xc[:], xc[:], g_sl, op=A.mult)
                    nc.vector.tensor_tensor(dst, xc[:], b_sl, op=A.add)

                for m in range(MPC):
                    pa = [None, None]
                    for h in range(H):
                        pa[h] = ops.tile([P, D], DT32, space="PSUM", tag="op",
                                         name=f"pa{h}")
                        nc.tensor.matmul(
                            pa[h][:],
                            lhsT=oT16[:, NPC * h + P * m:NPC * h + P * (m + 1)],
                            rhs=woT16[:, D * h:D * (h + 1)],
                            start=True, stop=True)
                    x1m = lsb.tile([P, D], DT32, tag="x1m")
                    nc.vector.tensor_scalar(x1m[:], pa[0][:],
                                            recT[:, m:m + 1], None,
                                            op0=A.mult)
                    nc.vector.scalar_tensor_tensor(
                        x1m[:], pa[1][:], recT[:, MPC + m:MPC + m + 1],
                        x1m[:], op0=A.mult, op1=A.add)
                    nc.vector.tensor_add(x1m[:], x1m[:],
                                         hbo[:, D * m:D * (m + 1)])
                    layernorm_m(h1_t[:, D * m:D * (m + 1)], x1m[:],
                                ln1g_bc, ln1b_bc, "a")
                    for f in range(2):
                        ptr = tps2.tile([P, P], DT32, space="PSUM", tag="tr2")
                        nc.tensor.transpose(
                            ptr[:],
                            h1_t[:, D * m + P * f:D * m + P * (f + 1)],
                            ident[:])
                        nc.vector.tensor_copy(
                            h1T16[:, NPC * f + P * m:NPC * f + P * (m + 1)],
                            ptr[:])

                # ---- FFN ----
                ff1T = ak.tile([P, (DFF // P) * NPC], DT16)
                with tc.tile_pool(name="f1_ps", bufs=3, space="PSUM") as fps:
                    for dc in range(DFF // P):
                        pf = fps.tile([P, NPC], DT32, space="PSUM", tag="f1")
                        for k in range(2):
                            nc.tensor.matmul(
                                pf[:],
                                lhsT=w1T16[:, DFF * k + P * dc:
                                           DFF * k + P * (dc + 1)],
                                rhs=h1T16[:, NPC * k:NPC * (k + 1)],
                                start=(k == 0), stop=(k == 1))
                        nc.scalar.activation(
                            ff1T[:, NPC * dc:NPC * (dc + 1)], pf[:], F.Relu,
                            bias=b1t[:, dc:dc + 1])

                with tc.tile_pool(name="f2_ps", bufs=2, space="PSUM") as fps2:
                    for m in range(MPC):
                        pf2 = fps2.tile([P, D], DT32, space="PSUM", tag="f2")
                        for kt2 in range(DFF // P):
                            nc.tensor.matmul(
                                pf2[:],
                                lhsT=ff1T[:, NPC * kt2 + P * m:
                                          NPC * kt2 + P * (m + 1)],
                                rhs=w2T16[:, D * kt2:D * (kt2 + 1)],
                                start=(kt2 == 0), stop=(kt2 == DFF // P - 1))
                        x2m = lsb.tile([P, D], DT32, tag="x2m")
                        nc.vector.scalar_tensor_tensor(
                            x2m[:], pf2[:], 1.0,
                            h1_t[:, D * m:D * (m + 1)], op0=A.mult, op1=A.add)
                        nc.vector.tensor_tensor(x2m[:], x2m[:], b2_bc,
                                                op=A.add)
                        out_m = lsb.tile([P, D], DT32, tag="outm")
                        layernorm_m(out_m[:], x2m[:], ln2g_bc, ln2b_bc, "b")
                        nc.scalar.dma_start(
                            out_d[:].rearrange("(m p) d -> m p d", p=P)[m],
                            out_m[:])
            ctx_attn.__exit__(None, None, None)

    nc.compile()
    return nc


# ======================= host-side prep =======================

def _prep_inputs(x, edge_index, edge_weight, W_gcn, b_gcn, in_proj_w,
                 in_proj_b, out_proj_w, out_proj_b, lin1_w, lin1_b, lin2_w,
                 lin2_b, ln1_g, ln1_b, ln2_g, ln2_b):
    """Pure index-permutation / layout prep. Returns per-core input maps."""
    x = np.asarray(x, np.float32)
    src = np.asarray(edge_index[0], np.int64)
    dst = np.asarray(edge_index[1], np.int64)
    w = np.asarray(edge_weight, np.float32)

    def wrap128(a):
        # [n*128, m] -> [128, n*m] with col block t <- rows [128t, 128t+128)
        n = a.shape[0] // P
        return np.ascontiguousarray(
            a.reshape(n, P, a.shape[1]).transpose(1, 0, 2).reshape(P, -1))

    ident = np.eye(P, dtype=np.float32)
    # out_proj bias with the V-bias term folded in (softmax rows sum to 1)
    bv = np.asarray(in_proj_b, np.float32)[2 * D:3 * D]
    bo_eff = (np.asarray(out_proj_b, np.float32)
              + bv @ np.asarray(out_proj_w, np.float32).T)
    bias_row = np.concatenate([
        np.asarray(v, np.float32).reshape(-1) for v in
        (b_gcn, lin2_b, ln1_g, ln1_b, ln2_g, ln2_b, bo_eff)
    ]).reshape(1, -1)
    bias_stack = np.ascontiguousarray(np.tile(bias_row, (P, 1)))

    f16 = np.float16
    shared = {
        "wg": wrap128(np.asarray(W_gcn, np.float32)).astype(f16),
        "ident": ident,
        "winT": wrap128(np.ascontiguousarray(
            np.asarray(in_proj_w, np.float32).T)).astype(f16),
        "ipb": np.ascontiguousarray(
            np.asarray(in_proj_b, np.float32).reshape(6, P).T),
        "woT": wrap128(np.ascontiguousarray(
            np.asarray(out_proj_w, np.float32).T)).astype(f16),
        "w1T": wrap128(np.ascontiguousarray(
            np.asarray(lin1_w, np.float32).T)).astype(f16),
        "b1": np.ascontiguousarray(
            np.asarray(lin1_b, np.float32).reshape(DFF // P, P).T),
        "w2T": wrap128(np.ascontiguousarray(
            np.asarray(lin2_w, np.float32).T)).astype(f16),
        "bias": bias_stack,
    }

    # j-major wrap of x.T: column block 512*j + 128*k holds the lhsT tile
    # for node block j, din block k -> xw[j] only needs its own 1 KB/row
    # prefix of the stream, so matmuls start while the DMA is in flight
    shared_xTf = np.ascontiguousarray(
        x.reshape(KT, P, DIN // P, P)          # [j, c, k, p]
         .transpose(3, 0, 2, 1)                # [p, j, k, c]
         .reshape(P, -1)).astype(f16)
    # full per-dst weight table for replicated degree computation
    wbdf = np.zeros((N, KBD), np.float32)
    cntf = np.zeros(N, np.int32)
    for di, wi in zip(dst.tolist(), w.tolist()):
        j = int(cntf[di])
        assert j < KBD
        wbdf[di, j] = wi
        cntf[di] = j + 1
    wbdf_full_w = wrap128(wbdf).astype(f16)

    core_of = dst // NPC
    in_maps = []
    for c in range(N_CORES):
        sel = np.nonzero(core_of == c)[0]
        s_c = src[sel]
        d_c = (dst[sel] - NPC * c).astype(np.int64)
        w_c = w[sel]

        w_arr = np.zeros((N, KPAD), np.float32)
        idx_arr = np.full((N, KPAD), -1, np.int16)
        counts = np.zeros(N, np.int32)
        first_slot = {}
        for si, di, wi in zip(s_c.tolist(), d_c.tolist(), w_c.tolist()):
            key = si * NPC + di
            slot = first_slot.get(key)
            if slot is None:
                j = int(counts[si])
                assert j < KPAD, f"KPAD overflow at src {si}"
                counts[si] = j + 1
                w_arr[si, j] = wi
                idx_arr[si, j] = di
                first_slot[key] = j
            else:
                # duplicate (src, dst) edge: fold its weight into the first
                # slot so the on-device scatter sees unique indices
                w_arr[si, slot] += wi

        wbd = np.zeros((NPC, KBD), np.float32)
        cnt2 = np.zeros(NPC, np.int32)
        for di, wi in zip(d_c.tolist(), w_c.tolist()):
            j = int(cnt2[di])
            assert j < KBD, f"KBD overflow at dst {di}"
            wbd[di, j] = wi
            cnt2[di] = j + 1

        in_maps.append({
            **shared,
            "xT": wrap128(np.ascontiguousarray(
                x[NPC * c:NPC * (c + 1)].T)).astype(f16),
            "xTf": shared_xTf,
            "wbdf": wbdf_full_w,
            "warr": wrap128(w_arr).astype(f16),
            "idx": wrap128(idx_arr),
            "wbd": wrap128(wbd).astype(f16),
        })
    return in_maps


# ======================= runner =======================

class _Runner:
    """Persistent-jit SPMD executor (mirrors bass2jax.run_bass_via_pjrt)."""

    def __init__(self, nc):
        import jax
        from jax.sharding import Mesh, PartitionSpec
        from jax.experimental.shard_map import shard_map
        from concourse.bass2jax import (_bass_exec_p, install_neuronx_cc_hook,
                                        partition_id_tensor)
        install_neuronx_cc_hook()
        self.jax = jax
        partition_name = (nc.partition_id_tensor.name
                          if nc.partition_id_tensor else None)
        in_names, out_names, out_avals, zero_outs = [], [], [], []
        for alloc in nc.m.functions[0].allocations:
            if not isinstance(alloc, mybir.MemoryLocationSet):
                continue
            name = alloc.memorylocations[0].name
            if alloc.kind == "ExternalInput":
                if name != partition_name:
                    in_names.append(name)
            elif alloc.kind == "ExternalOutput":
                out_names.append(name)
                shape = tuple(alloc.tensor_shape)
                dtype = mybir.dt.np(alloc.dtype)
                out_avals.append(jax.core.ShapedArray(shape, dtype))
                zero_outs.append(np.zeros(shape, dtype))
        self.in_names, self.out_names = in_names, out_names
        self.out_shapes = [tuple(a.shape) for a in out_avals]
        self.n_params = len(in_names)
        self.zero_outs = zero_outs
        all_in = in_names + out_names
        if partition_name is not None:
            all_in.append(partition_name)

        def _body(*args):
            operands = list(args)
            if partition_name is not None:
                operands.append(partition_id_tensor())
            return tuple(_bass_exec_p.bind(
                *operands, out_avals=tuple(out_avals), in_names=tuple(all_in),
                out_names=tuple(out_names), lowering_input_output_aliases=(),
                sim_require_finite=True, sim_require_nnan=True, nc=nc))

        devices = jax.devices()[:N_CORES]
        self.mesh = Mesh(np.asarray(devices), ("core",))
        nin = self.n_params + len(out_names)
        self.fn = jax.jit(
            shard_map(_body, mesh=self.mesh,
                      in_specs=(PartitionSpec("core"),) * nin,
                      out_specs=(PartitionSpec("core"),) * len(out_names),
                      check_rep=False),
            keep_unused=True)

    def place(self, in_maps):
        import jax
        from jax.sharding import PartitionSpec
        per_core = [[np.asarray(m[n]) for n in self.in_names] for m in in_maps]
        concat = [np.concatenate([per_core[c][i] for c in range(N_CORES)], axis=0)
                  for i in range(self.n_params)]
        zeros = [np.zeros((N_CORES * z.shape[0], *z.shape[1:]), z.dtype)
                 for z in self.zero_outs]
        sh = jax.sharding.NamedSharding(self.mesh, PartitionSpec("core"))
        return [jax.device_put(a, sh) for a in (*concat, *zeros)]

    def run(self, args):
        outs = self.fn(*args)
        self.jax.block_until_ready(outs)
        return outs

    def results(self, outs):
        res = []
        for c in range(N_CORES):
            d = {}
            for i, name in enumerate(self.out_names):
                full = np.asarray(outs[i])
                ps = self.out_shapes[i]
                d[name] = full.reshape((N_CORES,) + ps)[c]
            res.append(d)
        return res


_CACHE = {}


def _get_runner():
    if "runner" not in _CACHE:
        nc = build_kernel()
        _CACHE["nc"] = nc
        _CACHE["runner"] = _Runner(nc)
    return _CACHE["runner"]


def kernel(**inputs) -> np.ndarray:
    runner = _get_runner()
    in_maps = _prep_inputs(**inputs)
    args = runner.place(in_maps)
    outs = runner.run(args)
    res = runner.results(outs)
    return np.concatenate([res[c]["out"] for c in range(N_CORES)], axis=0)
